# revision 106
# baseline (speedup 1.0000x reference)
"""Trainium2 Bass kernel for the FFTBlock problem (B=2, C=32, H=2688, W=128).

Math (reference):
  spatial  = relu(conv7x1_s7(x) + b_spatial)                        [B,C,384,W]
  spectral = irfft(relu(w_spectral @ rfft_concat(x) + b_spectral))  per (b,c,w)
  out = spatial + spectral

rfft/irfft along H are linear, so with F the real-ified rfft matrix and G the
irfft matrix (dead rows/cols dropped):
  spectral_col = G @ relu(A @ x_col + b),   A = w_spectral @ F  [384, 2688]

Device plan (W sharded 8 x 16 columns, one launch per core):
  GEMM1  conv[384, 1024] = A @ x_cols: 10 DoubleRow steps (K=256, both
         operands e4m3, x64 A scale) + one [64, 2, *] packed DR tail step.
  relu   (ACT/DVE, bias, n-split so GEMM2 starts per-half) -> f16 (x64;
         G absorbs the scale)
  GEMM2  spec[384, 1024] = (G*32/64) @ relu  (f16, 3 k-tiles); the psum is
         copied to f16 SBUF on the otherwise-idle ACT.
  spatial conv TRANSPOSED: stationary = x chunks [(c,t)=224+bias row,
         h'-block], moving = w_spatial (e4m3 x32, bias folded as an extra
         contraction row), DoubleRow -> psum [h', (b,w,co)] -- same layout as
         spec, so the add happens on-device and only ONE output is stored.
  out = relu(spatial psum) + spec(sbuf) in ONE DVE scalar_tensor_tensor per
         chunk, f16 x32-scaled (host divides by 32); three grouped stores.

Column order everywhere is (b, w, c) so the spatial conv's 32-channel output
blocks are contiguous in the spectral column space. All DRAM layouts are
pre-swizzled on host to partition-major so every DMA moves >=512B runs; the
load order is tuned so the PE never starves and the last input (xsp b1 m2=2)
has the shortest dependent chain.
"""

import os

import numpy as np
import ml_dtypes

import concourse.bacc as bacc
import concourse.mybir as mybir
import concourse.tile as tile
from concourse.bass_utils import run_bass_kernel_spmd
from concourse.alu_op_type import AluOpType

N_CORES = 8
B, C, H, W = 2, 32, 2688, 128
FREQ_IN = H // 2 + 1            # 1345
OUT_H = 384
FREQ_OUT = OUT_H // 2 + 1       # 193
MO = 2 * FREQ_OUT - 2           # 384 usable conv channels
WS = W // N_CORES               # 16 width columns per core
NCOL = B * WS * C               # 1024 spectral columns per core, (b, w, c)
NSP = B * OUT_H * WS            # 12288 spatial cols (b, h', w)
OLDK = H // 128                 # 21 k-tiles of 128
KD_DR = 10                      # full DoubleRow steps (old-k 0..2*KD_DR-1)
# the odd tail tile (old-k 20) runs as a [64, 2, *] DoubleRow step
MT = 3                          # 128-row m-tiles (G1 out / G2 out)
NT = 2                          # 512-col n halves; n == b
KH = 112                        # (c,t) DR half-pairs for spatial
KHB = KH + 1                    # +1 bias row

AT_SCALE = 64.0                 # fp8 range helper for A = w_spec @ F
WSP_SCALE = 32.0                # fp8 range helper for the tiny spatial weights

F32 = mybir.dt.float32
F16 = mybir.dt.float16
F8E4 = mybir.dt.float8e4
F8E3 = mybir.dt.float8e3
RELU = mybir.ActivationFunctionType.Relu
DR = mybir.MatmulPerfMode.DoubleRow
E4M3 = ml_dtypes.float8_e4m3
E3M4 = ml_dtypes.float8_e3m4

_cache = {}
LAST_EXEC_NS = None

# scheduling knobs (tuned against TimelineSim; see tuner.py)
CFG = {
    "warms": 0,           # p-state bridge dummies between G1 and G2
    "dve_relu01_first": False,  # relu(0,1) before relu(1,0) on DVE
    # mid-stream load order (xsp groups always follow)
    "load_order": "a0 x0 x1 x2 a3 x3 gt x4 bs x5 ws x6 a7 x7 x8 x9 tl",
}


def _dft_constants():
    """F [2688, 2688] (rfft, ortho, dead rows dropped) and G [384, 384]
    (irfft, ortho, dead cols dropped)."""
    if "F" in _cache:
        return _cache["F"], _cache["G"]
    Fc = np.fft.rfft(np.eye(H), axis=0, norm="ortho")       # [1345, 2688]
    F = np.concatenate([Fc.real, Fc.imag[1:FREQ_IN - 1]], axis=0)
    F = np.ascontiguousarray(F, dtype=np.float32)           # [2688, 2688]
    G_re = np.fft.irfft(np.eye(FREQ_OUT), n=OUT_H, axis=0, norm="ortho")
    G_im = np.fft.irfft(1j * np.eye(FREQ_OUT), n=OUT_H, axis=0, norm="ortho")
    G = np.concatenate([G_re, G_im[:, 1:FREQ_OUT - 1]], axis=1)
    G = np.ascontiguousarray(G, dtype=np.float32)           # [384, 384]
    _cache["F"] = F
    _cache["G"] = G
    return F, G


def _spec_keep_idx():
    keep_f = list(range(FREQ_IN)) + [FREQ_IN + k for k in range(1, FREQ_IN - 1)]
    keep_o = list(range(FREQ_OUT)) + [FREQ_OUT + k for k in range(1, FREQ_OUT - 1)]
    return np.array(keep_f), np.array(keep_o)


def _build_main():
    key = ("main", repr(sorted(CFG.items())))
    if key in _cache:
        return _cache[key]
    nc = bacc.Bacc("TRN2", target_bir_lowering=False, debug=False,
                   num_devices=N_CORES)
    at = nc.dram_tensor("at", [128, KD_DR * 2 * MO], F8E4,
                        kind="ExternalInput").ap()
    xt = nc.dram_tensor("xt", [128, KD_DR * 2 * NCOL], F8E4,
                        kind="ExternalInput").ap()
    # packed DR tail: att [64, 768] and xtt [64, 2048] merged in one tensor
    tt = nc.dram_tensor("tt", [64, 2 * MO + 2 * NCOL], F8E4,
                        kind="ExternalInput").ap()
    gt = nc.dram_tensor("gt", [128, MT * MO], F16, kind="ExternalInput").ap()
    bspec = nc.dram_tensor("bspec", [128, MT], F32, kind="ExternalInput").ap()
    wsp = nc.dram_tensor("wsp", [KHB, 2 * C], F8E4, kind="ExternalInput").ap()
    xsp = nc.dram_tensor("xsp", [KHB, 2 * NSP], F8E4,
                         kind="ExternalInput").ap()
    out_d = nc.dram_tensor("out", [128, MT * NCOL], F16,
                           kind="ExternalOutput").ap()

    with tile.TileContext(nc) as tc:
        with tc.tile_pool(name="const", bufs=1) as cst, \
             tc.tile_pool(name="atp", bufs=1) as atp, \
             tc.tile_pool(name="xtp", bufs=1) as xtp, \
             tc.tile_pool(name="xspp", bufs=1) as xspp, \
             tc.tile_pool(name="relu", bufs=1) as rlp, \
             tc.tile_pool(name="outp", bufs=1) as outp, \
             tc.tile_pool(name="ps", bufs=1, space="PSUM") as psp, \
             tc.tile_pool(name="psw", bufs=1, space="PSUM") as psw:

            # ---------------- SBUF tiles ----------------
            AGRP = CFG.get("agrp", [(0, 3), (3, 7), (7, KD_DR)])
            at_g = {}
            for g0, g1 in AGRP:
                at_g[g0] = atp.tile([128, (g1 - g0) * 2 * MO], F8E4,
                                    tag=f"at{g0}", name=f"at{g0}")
            xt_t = [xtp.tile([128, 2 * NCOL], F8E4, tag=f"xt{s}",
                             name=f"xt{s}") for s in range(KD_DR)]
            tt_sb = xtp.tile([64, 2 * MO + 2 * NCOL], F8E4, tag="tt",
                             name="tt")
            gt_sb = cst.tile([128, MT * MO], F16, tag="gt", name="gt")
            bspec_sb = cst.tile([128, MT], F32, tag="bspec", name="bspec")
            wsp_sb = cst.tile([KHB, 2 * C], F8E4, tag="wsp", name="wsp")
            xsp_g = {(b, m2): xspp.tile([KHB, 2 * 2048], F8E4,
                                        tag=f"xsp{b}{m2}", name=f"xsp{b}{m2}")
                     for b in range(B) for m2 in range(MT)}
            # one tile per independently produced/consumed chunk -- the tile
            # framework tracks dependencies at TILE granularity, so shared
            # tiles would serialize unrelated producers/consumers
            relu_h = {(m, n): rlp.tile([128, 512], F16, tag=f"relu{m}{n}",
                                       name=f"relu{m}{n}")
                      for m in range(MT) for n in range(NT)}
            spec_sb = {(m2, n): rlp.tile([128, 512], F16, tag=f"sc{m2}{n}",
                                         name=f"sc{m2}{n}")
                       for m2 in range(MT) for n in range(NT)}
            # output SBUF grouped by store: A = all n0 chunks (ready ~same
            # time), B = (0,1)+(1,1), C = (2,1) alone -- three stores instead
            # of six halves the HWDGE generation serialization at the tail
            out_A = outp.tile([128, 1536], F16, tag="oA", name="oA")
            out_B = outp.tile([128, 1024], F16, tag="oB", name="oB")
            out_C = outp.tile([128, 512], F16, tag="oC", name="oC")

            def out_slice(m2, n):
                if n == 0:
                    return out_A[:, 512 * m2:512 * (m2 + 1)]
                if m2 == 2:
                    return out_C[:]
                return out_B[:, 512 * m2:512 * (m2 + 1)]

            def at_s(s, m):
                """DR stationary [*, 2, 128] for DR step s, m-tile m."""
                if s == KD_DR:
                    v = tt_sb[:, 0:2 * MO].rearrange("p (i m) -> p i m", i=2)
                else:
                    g0 = max(g for g, _ in AGRP if g <= s)
                    off = (s - g0) * 2 * MO
                    v = at_g[g0][:, off:off + 2 * MO].rearrange(
                        "p (i m) -> p i m", i=2)
                return v[:, :, 128 * m:128 * (m + 1)]

            def xt_s(s, n):
                if s == KD_DR:
                    v = tt_sb[:, 2 * MO:].rearrange("p (i n) -> p i n", i=2)
                else:
                    v = xt_t[s][:].rearrange("p (i n) -> p i n", i=2)
                return v[:, :, 512 * n:512 * (n + 1)]

            wsp_v = wsp_sb[:].rearrange("p (i m) -> p i m", i=2)
            gt_km = lambda k, m2: gt_sb[:, k * MO + 128 * m2:
                                        k * MO + 128 * (m2 + 1)]

            # ---------------- DMA emission (sync queue, in order) --------
            def load_at(g0):
                g1 = dict(AGRP)[g0]
                nc.sync.dma_start(at_g[g0][:],
                                  at[:, g0 * 2 * MO:g1 * 2 * MO])

            def load_xt(s):
                nc.sync.dma_start(xt_t[s][:],
                                  xt[:, s * 2 * NCOL:(s + 1) * 2 * NCOL])

            def load_tail():
                nc.sync.dma_start(tt_sb[:], tt[:])

            def load_xsp(b, m2):
                src = xsp.rearrange("p (i n) -> p i n", i=2)[
                    :, :, b * (OUT_H * WS) + m2 * 2048:
                    b * (OUT_H * WS) + (m2 + 1) * 2048]
                dst = xsp_g[(b, m2)][:].rearrange("p (i n) -> p i n", i=2)
                nc.sync.dma_start(dst, src)

            # order tuned so the PE (starting at xt0+900ns sem prop) never
            # starves; small consts hide mid-stream behind big transfers; the
            # last input (xsp b1 m2=2) has the shortest dependent chain
            loaders = {
                # a0/a3/a7 historically named; they mean AGRP groups 0/1/2
                "a0": lambda: load_at(AGRP[0][0]),
                "a3": lambda: load_at(AGRP[1][0]),
                "a7": lambda: load_at(AGRP[2][0]) if len(AGRP) > 2 else None,
                "tl": load_tail,
                "gt": lambda: nc.sync.dma_start(gt_sb[:], gt[:]),
                "bs": lambda: nc.sync.dma_start(bspec_sb[:], bspec[:]),
                "ws": lambda: nc.sync.dma_start(wsp_sb[:], wsp[:]),
            }
            for tok in CFG["load_order"].split():
                if tok.startswith("x"):
                    load_xt(int(tok[1:]))
                else:
                    loaders[tok]()
            for b, m2 in CFG.get("xsp_order",
                                 [(0, 0), (0, 1), (0, 2),
                                  (1, 0), (1, 1), (1, 2)]):
                load_xsp(b, m2)

            # ---------------- compute ----------------
            ps1 = {(m, n): psp.tile([128, 512], F32, tag=f"g1m{m}n{n}",
                                    name=f"g1m{m}n{n}")
                   for m in range(MT) for n in range(NT)}

            def g1_step(s):
                # s == KD_DR is the [64, 2, *] packed tail step
                for n in range(NT):
                    for m in range(MT):
                        nc.tensor.matmul(ps1[(m, n)][:], at_s(s, m),
                                         xt_s(s, n), start=(s == 0),
                                         stop=(s == KD_DR), perf_mode=DR)

            sp_ps = {}

            def sp_chunk(b, m2):
                # transposed spatial conv: stationary = x slices, moving = w.
                # 16 tiny DR matmuls land [h'-block, (w,co)] directly in the
                # spectral output layout.
                j = b * MT + m2
                ps = psw.tile([128, 512], F32, tag=f"spp{j % 2}",
                              name=f"spp{j}")
                sp_ps[(b, m2)] = ps
                xv = xsp_g[(b, m2)][:].rearrange("p (i n) -> p i n", i=2)
                for w in range(WS):
                    nc.tensor.matmul(ps[:, 32 * w:32 * (w + 1)],
                                     xv[:, :, w::WS], wsp_v,
                                     start=True, stop=True, perf_mode=DR)

            # n-split relus let g2(n0) start as soon as the last k-step's
            # (m, n0) groups close; ACT/DVE checkerboard
            RELU_ACT = {(0, 0), (2, 0), (1, 1)}

            def relu_m(m, n):
                # relu1 scaled x64 (G absorbs /64); only ACT/DVE read PSUM
                if (m, n) in RELU_ACT:
                    nc.scalar.activation(relu_h[(m, n)][:], ps1[(m, n)][:],
                                         RELU, bias=bspec_sb[:, m:m + 1])
                else:
                    nc.vector.tensor_scalar(relu_h[(m, n)][:], ps1[(m, n)][:],
                                            bspec_sb[:, m:m + 1], 0.0,
                                            AluOpType.add, AluOpType.max)

            ps2 = {}

            def g2_n(n):
                for m2 in range(MT):
                    ps2[(m2, n)] = psp.tile([128, 512], F32,
                                            tag=f"g1m{m2}n{n}",
                                            name=f"g2m{m2}n{n}")
                for k in range(MT):
                    for m2 in range(MT):
                        nc.tensor.matmul(ps2[(m2, n)][:], gt_km(k, m2),
                                         relu_h[(k, n)][:],
                                         start=(k == 0), stop=(k == MT - 1))

            def copy_spec(m2, n):
                # spec psum -> f16 sbuf on the otherwise-idle ACT, well
                # before the spatial psum arrives -- keeps the tail chain to
                # a single DVE op per chunk
                nc.scalar.activation(spec_sb[(m2, n)][:], ps2[(m2, n)][:],
                                     mybir.ActivationFunctionType.Copy)

            def fadd(m2, n):
                # out = relu(spatial psum) + spec(sbuf) in ONE DVE op; both
                # branches are x32 scaled (the host divides the output by 32)
                nc.vector.scalar_tensor_tensor(
                    out_slice(m2, n), sp_ps[(n, m2)][:], 0.0,
                    spec_sb[(m2, n)][:], AluOpType.max, AluOpType.add)

            # out DRAM columns are completion-ordered: [n0m0 n0m1 n0m2
            # n1m0 n1m1 n1m2]; the host unshard accounts for this
            def store_A():
                nc.sync.dma_start(out_d[:, 0:1536], out_A[:])

            def store_B():
                nc.sync.dma_start(out_d[:, 1536:2560], out_B[:])

            def store_C():
                nc.sync.dma_start(out_d[:, 2560:3072], out_C[:])

            def pe_warm(i):
                # tiny dummy matmul bridging the relu-latency gap between
                # GEMM1's last k-step and GEMM2, so the PE p-state (and with
                # it GEMM2's 2.4GHz rate) survives the wait. Reads the LAST
                # xt tile so the scheduler cannot hoist it earlier.
                ps = psw.tile([1, 256], F32, tag="spp0", name=f"warm{i}")
                nc.tensor.matmul(ps[:], xt_t[KD_DR - 1][:, 0:1],
                                 xt_t[KD_DR - 1][:, 0:256],
                                 start=True, stop=True)

            # ---- PE order ----
            for s in range(KD_DR + 1):
                g1_step(s)
            relu_m(0, 0)
            if CFG["dve_relu01_first"]:
                relu_m(0, 1)
                relu_m(2, 0)
                relu_m(1, 0)
            else:
                relu_m(1, 0)
                relu_m(2, 0)
                relu_m(0, 1)
            relu_m(1, 1)
            relu_m(2, 1)
            for i in range(CFG["warms"]):
                pe_warm(i)
            g2_n(0)
            sp_chunk(0, 0)
            sp_chunk(0, 1)
            sp_chunk(0, 2)
            copy_spec(0, 0)
            copy_spec(1, 0)
            copy_spec(2, 0)
            g2_n(1)
            if CFG.get("sc21_first"):
                copy_spec(2, 1)
                copy_spec(0, 1)
                copy_spec(1, 1)
            else:
                copy_spec(0, 1)
                copy_spec(1, 1)
                copy_spec(2, 1)
            fadd(0, 0)
            fadd(1, 0)
            fadd(2, 0)
            store_A()
            sp_chunk(1, 0)
            if CFG.get("fadd21_first"):
                sp_chunk(1, 1)
                sp_chunk(1, 2)
                fadd(2, 1)
                store_C()
                fadd(0, 1)
                fadd(1, 1)
                store_B()
            else:
                fadd(0, 1)
                sp_chunk(1, 1)
                fadd(1, 1)
                store_B()
                sp_chunk(1, 2)
                fadd(2, 1)
                store_C()

    nc.compile()
    _cache["main"] = nc
    return nc


def _host_prep(x, w_spatial, b_spatial, w_spectral, b_spectral):
    """Shared (weight) swizzles."""
    F, G = _dft_constants()
    keep_f, keep_o = _spec_keep_idx()

    A = w_spectral[keep_o][:, keep_f] @ F                    # [384, 2688]
    arrA = np.ascontiguousarray((A * AT_SCALE).T)            # [2688, 384]
    hcut = KD_DR * 256
    at_np = np.ascontiguousarray(
        arrA[:hcut].reshape(KD_DR * 2, 128, MO).transpose(1, 0, 2)
        .reshape(128, KD_DR * 2 * MO)).astype(E4M3)
    att_np = (arrA[hcut:].reshape(2, 64, MO).transpose(1, 0, 2)
              .reshape(64, 2 * MO)).astype(E4M3)
    # gt absorbs both the relu1 x64 scale and the x32 output scale (the
    # device emits 32*(spatial+spectral); the host divides by 32)
    gt_np = np.ascontiguousarray(
        (G.T * (WSP_SCALE / AT_SCALE)).reshape(MT, 128, MO).transpose(1, 0, 2)
        .reshape(128, MT * MO)).astype(np.float16)
    bspec_np = np.ascontiguousarray(
        (b_spectral[keep_o] * AT_SCALE).reshape(MT, 128).T).astype(np.float32)
    wbase = (w_spatial[:, :, :, 0].transpose(1, 2, 0).reshape(C * 7, C)
             * WSP_SCALE)
    wsp_np = np.concatenate([
        wbase.reshape(2, KH, C).transpose(1, 0, 2).reshape(KH, 2 * C),
        np.concatenate([b_spatial * WSP_SCALE, np.zeros(C)])[None, :],
    ], axis=0).astype(E4M3)                                  # [113, 64]
    return at_np, att_np, gt_np, bspec_np, wsp_np


def kernel(x, w_spatial, b_spatial, w_spectral, b_spectral):
    x = np.ascontiguousarray(x, dtype=np.float32)
    w_spatial = np.asarray(w_spatial, dtype=np.float32)
    b_spatial = np.asarray(b_spatial, dtype=np.float32)
    w_spectral = np.asarray(w_spectral, dtype=np.float32)
    b_spectral = np.asarray(b_spectral, dtype=np.float32)

    at_np, att_np, gt_np, bspec_np, wsp_np = _host_prep(
        x, w_spatial, b_spatial, w_spectral, b_spectral)
    core_ids = list(range(N_CORES))
    hcut = KD_DR * 256

    in_maps = []
    for i in core_ids:
        xs = x[:, :, :, WS * i:WS * (i + 1)]                 # [B, C, H, WS]
        arr = xs.transpose(2, 0, 3, 1).reshape(H, NCOL)      # [H, (b,w,c)]
        xt_np = np.ascontiguousarray(
            arr[:hcut].reshape(KD_DR * 2, 128, NCOL).transpose(1, 0, 2)
            .reshape(128, KD_DR * 2 * NCOL)).astype(E4M3)
        xtt_np = (arr[hcut:].reshape(2, 64, NCOL).transpose(1, 0, 2)
                  .reshape(64, 2 * NCOL)).astype(E4M3)
        tt_np = np.ascontiguousarray(
            np.concatenate([att_np, xtt_np], axis=1))        # [64, 2816]
        spbase = (xs.reshape(B, C, OUT_H, 7, WS).transpose(1, 3, 0, 2, 4)
                  .reshape(C * 7, NSP))                      # [(c,t),(b,h',w)]
        xsp_np = np.concatenate([
            spbase.reshape(2, KH, NSP).transpose(1, 0, 2).reshape(KH, 2 * NSP),
            np.concatenate([np.ones(NSP, np.float32),
                            np.zeros(NSP, np.float32)])[None, :],
        ], axis=0).astype(E4M3)                              # [113, 2*NSP]
        in_maps.append({"at": at_np, "xt": xt_np, "tt": tt_np,
                        "gt": gt_np, "bspec": bspec_np, "wsp": wsp_np,
                        "xsp": xsp_np})

    nc = _build_main()
    kw = {}
    if bool(int(os.environ.get("KERNEL_TRACE", "0"))):
        d = os.environ.get("KERNEL_TRACE_DIR", "/tmp/ktrace") + "/main"
        os.makedirs(d, exist_ok=True)
        kw = dict(trace=True, tmpdir=d)
    res = run_bass_kernel_spmd(nc, in_maps, core_ids, **kw)
    global LAST_EXEC_NS
    LAST_EXEC_NS = res.exec_time_ns

    # ---- host: unshard + undo the x32 device scale; device columns are
    # completion-ordered [n, m2] ----
    out = np.empty((B, C, OUT_H, W), np.float32)
    for i in core_ids:
        o = (res.results[i]["out"].astype(np.float32)
             .reshape(128, NT, MT, WS, C).transpose(1, 4, 2, 0, 3)
             .reshape(B, C, OUT_H, WS))
        out[:, :, :, WS * i:WS * (i + 1)] = o * (1.0 / WSP_SCALE)
    return out


# revision 113
# speedup vs baseline: 1.0024x; 1.0024x over previous
"""Trainium2 Bass kernel for the FFTBlock problem (B=2, C=32, H=2688, W=128).

Math (reference):
  spatial  = relu(conv7x1_s7(x) + b_spatial)                        [B,C,384,W]
  spectral = irfft(relu(w_spectral @ rfft_concat(x) + b_spectral))  per (b,c,w)
  out = spatial + spectral

rfft/irfft along H are linear, so with F the real-ified rfft matrix and G the
irfft matrix (dead rows/cols dropped):
  spectral_col = G @ relu(A @ x_col + b),   A = w_spectral @ F  [384, 2688]

Device plan (W sharded 8 x 16 columns, one launch per core):
  GEMM1  conv[384, 1024] = A @ x_cols: 10 DoubleRow steps (K=256, both
         operands e4m3, x64 A scale) + one [64, 2, *] packed DR tail step.
  relu   (ACT/DVE, bias, n-split so GEMM2 starts per-half) -> f16 (x64;
         G absorbs the scale)
  GEMM2  spec[384, 1024] = (G*32/64) @ relu  (f16, 3 k-tiles); the psum is
         copied to f16 SBUF on the otherwise-idle ACT.
  spatial conv TRANSPOSED: stationary = x chunks [(c,t)=224+bias row,
         h'-block], moving = w_spatial (e4m3 x32, bias folded as an extra
         contraction row), DoubleRow -> psum [h', (b,w,co)] -- same layout as
         spec, so the add happens on-device and only ONE output is stored.
  out = relu(spatial psum) + spec(sbuf) in ONE DVE scalar_tensor_tensor per
         chunk, f16 x32-scaled (host divides by 32); three grouped stores.

Column order everywhere is (b, w, c) so the spatial conv's 32-channel output
blocks are contiguous in the spectral column space. All DRAM layouts are
pre-swizzled on host to partition-major so every DMA moves >=512B runs; the
load order is tuned so the PE never starves and the last input (xsp b1 m2=2)
has the shortest dependent chain.
"""

import os

import numpy as np
import ml_dtypes

import concourse.bacc as bacc
import concourse.mybir as mybir
import concourse.tile as tile
from concourse.bass_utils import run_bass_kernel_spmd
from concourse.alu_op_type import AluOpType

N_CORES = 8
B, C, H, W = 2, 32, 2688, 128
FREQ_IN = H // 2 + 1            # 1345
OUT_H = 384
FREQ_OUT = OUT_H // 2 + 1       # 193
MO = 2 * FREQ_OUT - 2           # 384 usable conv channels
WS = W // N_CORES               # 16 width columns per core
NCOL = B * WS * C               # 1024 spectral columns per core, (b, w, c)
NSP = B * OUT_H * WS            # 12288 spatial cols (b, h', w)
OLDK = H // 128                 # 21 k-tiles of 128
KD_DR = 10                      # full DoubleRow steps (old-k 0..2*KD_DR-1)
# the odd tail tile (old-k 20) runs as a [64, 2, *] DoubleRow step
MT = 3                          # 128-row m-tiles (G1 out / G2 out)
NT = 2                          # 512-col n halves; n == b
KH = 112                        # (c,t) DR half-pairs for spatial
KHB = KH + 1                    # +1 bias row

AT_SCALE = 64.0                 # fp8 range helper for A = w_spec @ F
WSP_SCALE = 32.0                # fp8 range helper for the tiny spatial weights

F32 = mybir.dt.float32
F16 = mybir.dt.float16
F8E4 = mybir.dt.float8e4
F8E3 = mybir.dt.float8e3
RELU = mybir.ActivationFunctionType.Relu
DR = mybir.MatmulPerfMode.DoubleRow
E4M3 = ml_dtypes.float8_e4m3
E3M4 = ml_dtypes.float8_e3m4

_cache = {}
LAST_EXEC_NS = None

# scheduling knobs (tuned against TimelineSim; see tuner.py)
CFG = {
    "warms": 0,           # p-state bridge dummies between G1 and G2
    "dve_relu01_first": False,  # relu(0,1) before relu(1,0) on DVE
    # mid-stream load order (xsp groups always follow)
    "load_order": "a0 x0 x1 x2 a3 x3 gt x4 bs x5 ws x6 a7 x7 x8 x9 tl",
}


def _dft_constants():
    """F [2688, 2688] (rfft, ortho, dead rows dropped) and G [384, 384]
    (irfft, ortho, dead cols dropped)."""
    if "F" in _cache:
        return _cache["F"], _cache["G"]
    Fc = np.fft.rfft(np.eye(H), axis=0, norm="ortho")       # [1345, 2688]
    F = np.concatenate([Fc.real, Fc.imag[1:FREQ_IN - 1]], axis=0)
    F = np.ascontiguousarray(F, dtype=np.float32)           # [2688, 2688]
    G_re = np.fft.irfft(np.eye(FREQ_OUT), n=OUT_H, axis=0, norm="ortho")
    G_im = np.fft.irfft(1j * np.eye(FREQ_OUT), n=OUT_H, axis=0, norm="ortho")
    G = np.concatenate([G_re, G_im[:, 1:FREQ_OUT - 1]], axis=1)
    G = np.ascontiguousarray(G, dtype=np.float32)           # [384, 384]
    _cache["F"] = F
    _cache["G"] = G
    return F, G


def _spec_keep_idx():
    keep_f = list(range(FREQ_IN)) + [FREQ_IN + k for k in range(1, FREQ_IN - 1)]
    keep_o = list(range(FREQ_OUT)) + [FREQ_OUT + k for k in range(1, FREQ_OUT - 1)]
    return np.array(keep_f), np.array(keep_o)


def _build_main():
    key = ("main", repr(sorted(CFG.items())))
    if key in _cache:
        return _cache[key]
    nc = bacc.Bacc("TRN2", target_bir_lowering=False, debug=False,
                   num_devices=N_CORES)
    at = nc.dram_tensor("at", [128, KD_DR * 2 * MO], F8E4,
                        kind="ExternalInput").ap()
    xt = nc.dram_tensor("xt", [128, KD_DR * 2 * NCOL], F8E4,
                        kind="ExternalInput").ap()
    # packed DR tail: att [64, 768] and xtt [64, 2048] merged in one tensor
    tt = nc.dram_tensor("tt", [64, 2 * MO + 2 * NCOL], F8E4,
                        kind="ExternalInput").ap()
    gt = nc.dram_tensor("gt", [128, MT * MO], F16, kind="ExternalInput").ap()
    bspec = nc.dram_tensor("bspec", [128, MT], F32, kind="ExternalInput").ap()
    wsp = nc.dram_tensor("wsp", [KHB, 2 * C], F8E4, kind="ExternalInput").ap()
    xsp = nc.dram_tensor("xsp", [KHB, 2 * NSP], F8E4,
                         kind="ExternalInput").ap()
    # last spatial group (b=1, m2=2) w-split into [12w | 4w] pieces so only
    # a small fadd trails the final input transfer
    xspL = nc.dram_tensor("xspL", [KHB, 2 * 2048], F8E4,
                          kind="ExternalInput").ap()
    out_d = nc.dram_tensor("out", [128, MT * NCOL], F16,
                           kind="ExternalOutput").ap()

    with tile.TileContext(nc) as tc:
        with tc.tile_pool(name="const", bufs=1) as cst, \
             tc.tile_pool(name="atp", bufs=1) as atp, \
             tc.tile_pool(name="xtp", bufs=1) as xtp, \
             tc.tile_pool(name="xspp", bufs=1) as xspp, \
             tc.tile_pool(name="relu", bufs=1) as rlp, \
             tc.tile_pool(name="outp", bufs=1) as outp, \
             tc.tile_pool(name="ps", bufs=1, space="PSUM") as psp, \
             tc.tile_pool(name="psw", bufs=1, space="PSUM") as psw:

            # ---------------- SBUF tiles ----------------
            AGRP = CFG.get("agrp", [(0, 3), (3, 7), (7, KD_DR)])
            at_g = {}
            for g0, g1 in AGRP:
                at_g[g0] = atp.tile([128, (g1 - g0) * 2 * MO], F8E4,
                                    tag=f"at{g0}", name=f"at{g0}")
            xt_t = [xtp.tile([128, 2 * NCOL], F8E4, tag=f"xt{s}",
                             name=f"xt{s}") for s in range(KD_DR)]
            tt_sb = xtp.tile([64, 2 * MO + 2 * NCOL], F8E4, tag="tt",
                             name="tt")
            gt_sb = cst.tile([128, MT * MO], F16, tag="gt", name="gt")
            bspec_sb = cst.tile([128, MT], F32, tag="bspec", name="bspec")
            wsp_sb = cst.tile([KHB, 2 * C], F8E4, tag="wsp", name="wsp")
            xsp_g = {(b, m2): xspp.tile([KHB, 2 * 2048], F8E4,
                                        tag=f"xsp{b}{m2}", name=f"xsp{b}{m2}")
                     for b in range(B) for m2 in range(MT)
                     if (b, m2) != (1, 2)}
            xspA = xspp.tile([KHB, 2 * 1536], F8E4, tag="xspA", name="xspA")
            xspB = xspp.tile([KHB, 2 * 512], F8E4, tag="xspB", name="xspB")
            # one tile per independently produced/consumed chunk -- the tile
            # framework tracks dependencies at TILE granularity, so shared
            # tiles would serialize unrelated producers/consumers
            relu_h = {(m, n): rlp.tile([128, 512], F16, tag=f"relu{m}{n}",
                                       name=f"relu{m}{n}")
                      for m in range(MT) for n in range(NT)}
            spec_sb = {(m2, n): rlp.tile([128, 512], F16, tag=f"sc{m2}{n}",
                                         name=f"sc{m2}{n}")
                       for m2 in range(MT) for n in range(NT)}
            # output SBUF grouped by store: A = all n0 chunks (ready ~same
            # time), B = (0,1)+(1,1), C = (2,1) alone -- three stores instead
            # of six halves the HWDGE generation serialization at the tail
            out_A = outp.tile([128, 1536], F16, tag="oA", name="oA")
            out_B = outp.tile([128, 1024], F16, tag="oB", name="oB")
            out_C = outp.tile([128, 512], F16, tag="oC", name="oC")

            def out_slice(m2, n):
                if n == 0:
                    return out_A[:, 512 * m2:512 * (m2 + 1)]
                if m2 == 2:
                    return out_C[:]
                return out_B[:, 512 * m2:512 * (m2 + 1)]

            def at_s(s, m):
                """DR stationary [*, 2, 128] for DR step s, m-tile m."""
                if s == KD_DR:
                    v = tt_sb[:, 0:2 * MO].rearrange("p (i m) -> p i m", i=2)
                else:
                    g0 = max(g for g, _ in AGRP if g <= s)
                    off = (s - g0) * 2 * MO
                    v = at_g[g0][:, off:off + 2 * MO].rearrange(
                        "p (i m) -> p i m", i=2)
                return v[:, :, 128 * m:128 * (m + 1)]

            def xt_s(s, n):
                if s == KD_DR:
                    v = tt_sb[:, 2 * MO:].rearrange("p (i n) -> p i n", i=2)
                else:
                    v = xt_t[s][:].rearrange("p (i n) -> p i n", i=2)
                return v[:, :, 512 * n:512 * (n + 1)]

            wsp_v = wsp_sb[:].rearrange("p (i m) -> p i m", i=2)
            gt_km = lambda k, m2: gt_sb[:, k * MO + 128 * m2:
                                        k * MO + 128 * (m2 + 1)]

            # ---------------- DMA emission (sync queue, in order) --------
            def load_at(g0):
                g1 = dict(AGRP)[g0]
                nc.sync.dma_start(at_g[g0][:],
                                  at[:, g0 * 2 * MO:g1 * 2 * MO])

            def load_xt(s):
                nc.sync.dma_start(xt_t[s][:],
                                  xt[:, s * 2 * NCOL:(s + 1) * 2 * NCOL])

            def load_tail():
                nc.sync.dma_start(tt_sb[:], tt[:])

            def load_xsp(b, m2):
                if (b, m2) == (1, 2):
                    srcL = xspL.rearrange("p (i n) -> p i n", i=2)
                    nc.sync.dma_start(
                        xspA[:].rearrange("p (i n) -> p i n", i=2),
                        srcL[:, :, 0:1536])
                    nc.sync.dma_start(
                        xspB[:].rearrange("p (i n) -> p i n", i=2),
                        srcL[:, :, 1536:2048])
                    return
                src = xsp.rearrange("p (i n) -> p i n", i=2)[
                    :, :, b * (OUT_H * WS) + m2 * 2048:
                    b * (OUT_H * WS) + (m2 + 1) * 2048]
                dst = xsp_g[(b, m2)][:].rearrange("p (i n) -> p i n", i=2)
                nc.sync.dma_start(dst, src)

            # order tuned so the PE (starting at xt0+900ns sem prop) never
            # starves; small consts hide mid-stream behind big transfers; the
            # last input (xsp b1 m2=2) has the shortest dependent chain
            loaders = {
                # a0/a3/a7 historically named; they mean AGRP groups 0/1/2
                "a0": lambda: load_at(AGRP[0][0]),
                "a3": lambda: load_at(AGRP[1][0]),
                "a7": lambda: load_at(AGRP[2][0]) if len(AGRP) > 2 else None,
                "tl": load_tail,
                "gt": lambda: nc.sync.dma_start(gt_sb[:], gt[:]),
                "bs": lambda: nc.sync.dma_start(bspec_sb[:], bspec[:]),
                "ws": lambda: nc.sync.dma_start(wsp_sb[:], wsp[:]),
            }
            for tok in CFG["load_order"].split():
                if tok.startswith("x"):
                    load_xt(int(tok[1:]))
                else:
                    loaders[tok]()
            for b, m2 in CFG.get("xsp_order",
                                 [(0, 0), (0, 1), (0, 2),
                                  (1, 0), (1, 1), (1, 2)]):
                load_xsp(b, m2)

            # ---------------- compute ----------------
            ps1 = {(m, n): psp.tile([128, 512], F32, tag=f"g1m{m}n{n}",
                                    name=f"g1m{m}n{n}")
                   for m in range(MT) for n in range(NT)}

            def g1_step(s):
                # s == KD_DR is the [64, 2, *] packed tail step
                for n in range(NT):
                    for m in range(MT):
                        nc.tensor.matmul(ps1[(m, n)][:], at_s(s, m),
                                         xt_s(s, n), start=(s == 0),
                                         stop=(s == KD_DR), perf_mode=DR)

            sp_ps = {}

            def sp_chunk(b, m2):
                # transposed spatial conv: stationary = x slices, moving = w.
                # 16 tiny DR matmuls land [h'-block, (w,co)] directly in the
                # spectral output layout.
                j = b * MT + m2
                ps = psw.tile([128, 512], F32, tag=f"spp{j % 2}",
                              name=f"spp{j}")
                sp_ps[(b, m2)] = ps
                xv = xsp_g[(b, m2)][:].rearrange("p (i n) -> p i n", i=2)
                for w in range(WS):
                    nc.tensor.matmul(ps[:, 32 * w:32 * (w + 1)],
                                     xv[:, :, w::WS], wsp_v,
                                     start=True, stop=True, perf_mode=DR)

            def sp_piece(which):
                # (1,2) split: separate psum tiles so piece A's add never
                # waits piece B's (later) DMA
                if which == 0:
                    ps = psw.tile([128, 384], F32, tag="spp1", name="sppA")
                    xv = xspA[:].rearrange("p (i n) -> p i n", i=2)
                    nw = 12
                else:
                    ps = psp.tile([128, 128], F32, tag="g1m0n0", name="sppB")
                    xv = xspB[:].rearrange("p (i n) -> p i n", i=2)
                    nw = 4
                sp_ps[("L", which)] = ps
                for w in range(nw):
                    nc.tensor.matmul(ps[:, 32 * w:32 * (w + 1)],
                                     xv[:, :, w::nw], wsp_v,
                                     start=True, stop=True, perf_mode=DR)

            # n-split relus let g2(n0) start as soon as the last k-step's
            # (m, n0) groups close; ACT/DVE checkerboard
            RELU_ACT = {(0, 0), (2, 0), (1, 1)}

            def relu_m(m, n):
                # relu1 scaled x64 (G absorbs /64); only ACT/DVE read PSUM
                if (m, n) in RELU_ACT:
                    nc.scalar.activation(relu_h[(m, n)][:], ps1[(m, n)][:],
                                         RELU, bias=bspec_sb[:, m:m + 1])
                else:
                    nc.vector.tensor_scalar(relu_h[(m, n)][:], ps1[(m, n)][:],
                                            bspec_sb[:, m:m + 1], 0.0,
                                            AluOpType.add, AluOpType.max)

            ps2 = {}

            def g2_n(n):
                for m2 in range(MT):
                    ps2[(m2, n)] = psp.tile([128, 512], F32,
                                            tag=f"g1m{m2}n{n}",
                                            name=f"g2m{m2}n{n}")
                if CFG.get("g2_m_major"):
                    # m-major: each m2 psum group closes ASAP, feeding the
                    # ACT spec-copy -> DVE fadd staircase earlier
                    for m2 in range(MT):
                        for k in range(MT):
                            nc.tensor.matmul(ps2[(m2, n)][:], gt_km(k, m2),
                                             relu_h[(k, n)][:],
                                             start=(k == 0),
                                             stop=(k == MT - 1))
                else:
                    for k in range(MT):
                        for m2 in range(MT):
                            nc.tensor.matmul(ps2[(m2, n)][:], gt_km(k, m2),
                                             relu_h[(k, n)][:],
                                             start=(k == 0),
                                             stop=(k == MT - 1))

            def copy_spec(m2, n):
                # spec psum -> f16 sbuf on the otherwise-idle ACT, well
                # before the spatial psum arrives -- keeps the tail chain to
                # a single DVE op per chunk
                nc.scalar.activation(spec_sb[(m2, n)][:], ps2[(m2, n)][:],
                                     mybir.ActivationFunctionType.Copy)

            def fadd(m2, n):
                # out = relu(spatial psum) + spec(sbuf) in ONE DVE op; both
                # branches are x32 scaled (the host divides the output by 32)
                nc.vector.scalar_tensor_tensor(
                    out_slice(m2, n), sp_ps[(n, m2)][:], 0.0,
                    spec_sb[(m2, n)][:], AluOpType.max, AluOpType.add)

            # out DRAM columns are completion-ordered: [n0m0 n0m1 n0m2
            # n1m0 n1m1 n1m2]; the host unshard accounts for this
            def store_A():
                nc.sync.dma_start(out_d[:, 0:1536], out_A[:])

            def store_B():
                nc.sync.dma_start(out_d[:, 1536:2560], out_B[:])

            def store_C():
                nc.sync.dma_start(out_d[:, 2560:3072], out_C[:])

            def pe_warm(i):
                # tiny dummy matmul bridging the relu-latency gap between
                # GEMM1's last k-step and GEMM2, so the PE p-state (and with
                # it GEMM2's 2.4GHz rate) survives the wait. Reads the LAST
                # xt tile so the scheduler cannot hoist it earlier.
                ps = psw.tile([1, 256], F32, tag="spp0", name=f"warm{i}")
                nc.tensor.matmul(ps[:], xt_t[KD_DR - 1][:, 0:1],
                                 xt_t[KD_DR - 1][:, 0:256],
                                 start=True, stop=True)

            # ---- PE order ----
            for s in range(KD_DR + 1):
                g1_step(s)
            relu_m(0, 0)
            if CFG["dve_relu01_first"]:
                relu_m(0, 1)
                relu_m(2, 0)
                relu_m(1, 0)
            else:
                relu_m(1, 0)
                relu_m(2, 0)
                relu_m(0, 1)
            relu_m(1, 1)
            relu_m(2, 1)
            for i in range(CFG["warms"]):
                pe_warm(i)
            g2_n(0)
            sp_chunk(0, 0)
            sp_chunk(0, 1)
            sp_chunk(0, 2)
            copy_spec(0, 0)
            copy_spec(1, 0)
            copy_spec(2, 0)
            g2_n(1)
            if CFG.get("sc21_first"):
                copy_spec(2, 1)
                copy_spec(0, 1)
                copy_spec(1, 1)
            else:
                copy_spec(0, 1)
                copy_spec(1, 1)
                copy_spec(2, 1)
            fadd(0, 0)
            fadd(1, 0)
            fadd(2, 0)
            store_A()
            sp_chunk(1, 0)
            fadd(0, 1)
            sp_chunk(1, 1)
            fadd(1, 1)
            store_B()
            sp_piece(0)
            nc.vector.scalar_tensor_tensor(
                out_C[:, 0:384], sp_ps[("L", 0)][:], 0.0,
                spec_sb[(2, 1)][:, 0:384], AluOpType.max, AluOpType.add)
            sp_piece(1)
            nc.vector.scalar_tensor_tensor(
                out_C[:, 384:512], sp_ps[("L", 1)][:], 0.0,
                spec_sb[(2, 1)][:, 384:512], AluOpType.max, AluOpType.add)
            store_C()

    nc.compile()
    _cache["main"] = nc
    return nc


def _host_prep(x, w_spatial, b_spatial, w_spectral, b_spectral):
    """Shared (weight) swizzles."""
    F, G = _dft_constants()
    keep_f, keep_o = _spec_keep_idx()

    A = w_spectral[keep_o][:, keep_f] @ F                    # [384, 2688]
    arrA = np.ascontiguousarray((A * AT_SCALE).T)            # [2688, 384]
    hcut = KD_DR * 256
    at_np = np.ascontiguousarray(
        arrA[:hcut].reshape(KD_DR * 2, 128, MO).transpose(1, 0, 2)
        .reshape(128, KD_DR * 2 * MO)).astype(E4M3)
    att_np = (arrA[hcut:].reshape(2, 64, MO).transpose(1, 0, 2)
              .reshape(64, 2 * MO)).astype(E4M3)
    # gt absorbs both the relu1 x64 scale and the x32 output scale (the
    # device emits 32*(spatial+spectral); the host divides by 32)
    gt_np = np.ascontiguousarray(
        (G.T * (WSP_SCALE / AT_SCALE)).reshape(MT, 128, MO).transpose(1, 0, 2)
        .reshape(128, MT * MO)).astype(np.float16)
    bspec_np = np.ascontiguousarray(
        (b_spectral[keep_o] * AT_SCALE).reshape(MT, 128).T).astype(np.float32)
    wbase = (w_spatial[:, :, :, 0].transpose(1, 2, 0).reshape(C * 7, C)
             * WSP_SCALE)
    wsp_np = np.concatenate([
        wbase.reshape(2, KH, C).transpose(1, 0, 2).reshape(KH, 2 * C),
        np.concatenate([b_spatial * WSP_SCALE, np.zeros(C)])[None, :],
    ], axis=0).astype(E4M3)                                  # [113, 64]
    return at_np, att_np, gt_np, bspec_np, wsp_np


def kernel(x, w_spatial, b_spatial, w_spectral, b_spectral):
    x = np.ascontiguousarray(x, dtype=np.float32)
    w_spatial = np.asarray(w_spatial, dtype=np.float32)
    b_spatial = np.asarray(b_spatial, dtype=np.float32)
    w_spectral = np.asarray(w_spectral, dtype=np.float32)
    b_spectral = np.asarray(b_spectral, dtype=np.float32)

    at_np, att_np, gt_np, bspec_np, wsp_np = _host_prep(
        x, w_spatial, b_spatial, w_spectral, b_spectral)
    core_ids = list(range(N_CORES))
    hcut = KD_DR * 256

    in_maps = []
    for i in core_ids:
        xs = x[:, :, :, WS * i:WS * (i + 1)]                 # [B, C, H, WS]
        arr = xs.transpose(2, 0, 3, 1).reshape(H, NCOL)      # [H, (b,w,c)]
        xt_np = np.ascontiguousarray(
            arr[:hcut].reshape(KD_DR * 2, 128, NCOL).transpose(1, 0, 2)
            .reshape(128, KD_DR * 2 * NCOL)).astype(E4M3)
        xtt_np = (arr[hcut:].reshape(2, 64, NCOL).transpose(1, 0, 2)
                  .reshape(64, 2 * NCOL)).astype(E4M3)
        tt_np = np.ascontiguousarray(
            np.concatenate([att_np, xtt_np], axis=1))        # [64, 2816]
        spbase = (xs.reshape(B, C, OUT_H, 7, WS).transpose(1, 3, 0, 2, 4)
                  .reshape(C * 7, NSP))                      # [(c,t),(b,h',w)]
        xsp_np = np.concatenate([
            spbase.reshape(2, KH, NSP).transpose(1, 0, 2).reshape(KH, 2 * NSP),
            np.concatenate([np.ones(NSP, np.float32),
                            np.zeros(NSP, np.float32)])[None, :],
        ], axis=0).astype(E4M3)                              # [113, 2*NSP]
        # last group (b=1, m2=2) w-split [12w | 4w], cols (w-block, h', w)
        grp = spbase[:, 5 * 2048:6 * 2048].reshape(C * 7, 128, WS)
        cat = np.concatenate([grp[:, :, :12].reshape(C * 7, 1536),
                              grp[:, :, 12:].reshape(C * 7, 512)], axis=1)
        xspL_np = np.concatenate([
            cat.reshape(2, KH, 2048).transpose(1, 0, 2).reshape(KH, 4096),
            np.concatenate([np.ones(2048, np.float32),
                            np.zeros(2048, np.float32)])[None, :],
        ], axis=0).astype(E4M3)                              # [113, 4096]
        in_maps.append({"at": at_np, "xt": xt_np, "tt": tt_np,
                        "gt": gt_np, "bspec": bspec_np, "wsp": wsp_np,
                        "xsp": xsp_np, "xspL": xspL_np})

    nc = _build_main()
    kw = {}
    if bool(int(os.environ.get("KERNEL_TRACE", "0"))):
        d = os.environ.get("KERNEL_TRACE_DIR", "/tmp/ktrace") + "/main"
        os.makedirs(d, exist_ok=True)
        kw = dict(trace=True, tmpdir=d)
    res = run_bass_kernel_spmd(nc, in_maps, core_ids, **kw)
    global LAST_EXEC_NS
    LAST_EXEC_NS = res.exec_time_ns

    # ---- host: unshard + undo the x32 device scale; device columns are
    # completion-ordered [n, m2] ----
    out = np.empty((B, C, OUT_H, W), np.float32)
    for i in core_ids:
        o = (res.results[i]["out"].astype(np.float32)
             .reshape(128, NT, MT, WS, C).transpose(1, 4, 2, 0, 3)
             .reshape(B, C, OUT_H, WS))
        out[:, :, :, WS * i:WS * (i + 1)] = o * (1.0 / WSP_SCALE)
    return out


# revision 114
# speedup vs baseline: 1.0066x; 1.0042x over previous
"""Trainium2 Bass kernel for the FFTBlock problem (B=2, C=32, H=2688, W=128).

Math (reference):
  spatial  = relu(conv7x1_s7(x) + b_spatial)                        [B,C,384,W]
  spectral = irfft(relu(w_spectral @ rfft_concat(x) + b_spectral))  per (b,c,w)
  out = spatial + spectral

rfft/irfft along H are linear, so with F the real-ified rfft matrix and G the
irfft matrix (dead rows/cols dropped):
  spectral_col = G @ relu(A @ x_col + b),   A = w_spectral @ F  [384, 2688]

Device plan (W sharded 8 x 16 columns, one launch per core):
  GEMM1  conv[384, 1024] = A @ x_cols: 10 DoubleRow steps (K=256, both
         operands e4m3, x64 A scale) + one [64, 2, *] packed DR tail step.
  relu   (ACT/DVE, bias, n-split so GEMM2 starts per-half) -> f16 (x64;
         G absorbs the scale)
  GEMM2  spec[384, 1024] = (G*32/64) @ relu  (f16, 3 k-tiles); the psum is
         copied to f16 SBUF on the otherwise-idle ACT.
  spatial conv TRANSPOSED: stationary = x chunks [(c,t)=224+bias row,
         h'-block], moving = w_spatial (e4m3 x32, bias folded as an extra
         contraction row), DoubleRow -> psum [h', (b,w,co)] -- same layout as
         spec, so the add happens on-device and only ONE output is stored.
  out = relu(spatial psum) + spec(sbuf) in ONE DVE scalar_tensor_tensor per
         chunk, f16 x32-scaled (host divides by 32); three grouped stores.

Column order everywhere is (b, w, c) so the spatial conv's 32-channel output
blocks are contiguous in the spectral column space. All DRAM layouts are
pre-swizzled on host to partition-major so every DMA moves >=512B runs; the
load order is tuned so the PE never starves and the last input (xsp b1 m2=2)
has the shortest dependent chain.
"""

import os

import numpy as np
import ml_dtypes

import concourse.bacc as bacc
import concourse.mybir as mybir
import concourse.tile as tile
from concourse.bass_utils import run_bass_kernel_spmd
from concourse.alu_op_type import AluOpType

N_CORES = 8
B, C, H, W = 2, 32, 2688, 128
FREQ_IN = H // 2 + 1            # 1345
OUT_H = 384
FREQ_OUT = OUT_H // 2 + 1       # 193
MO = 2 * FREQ_OUT - 2           # 384 usable conv channels
WS = W // N_CORES               # 16 width columns per core
NCOL = B * WS * C               # 1024 spectral columns per core, (b, w, c)
NSP = B * OUT_H * WS            # 12288 spatial cols (b, h', w)
OLDK = H // 128                 # 21 k-tiles of 128
KD_DR = 10                      # full DoubleRow steps (old-k 0..2*KD_DR-1)
# the odd tail tile (old-k 20) runs as a [64, 2, *] DoubleRow step
MT = 3                          # 128-row m-tiles (G1 out / G2 out)
NT = 2                          # 512-col n halves; n == b
KH = 112                        # (c,t) DR half-pairs for spatial
KHB = KH + 1                    # +1 bias row

AT_SCALE = 64.0                 # fp8 range helper for A = w_spec @ F
WSP_SCALE = 32.0                # fp8 range helper for the tiny spatial weights

F32 = mybir.dt.float32
F16 = mybir.dt.float16
F8E4 = mybir.dt.float8e4
F8E3 = mybir.dt.float8e3
RELU = mybir.ActivationFunctionType.Relu
DR = mybir.MatmulPerfMode.DoubleRow
E4M3 = ml_dtypes.float8_e4m3
E3M4 = ml_dtypes.float8_e3m4

_cache = {}
LAST_EXEC_NS = None

# scheduling knobs (tuned against TimelineSim; see tuner.py)
CFG = {
    "warms": 0,           # p-state bridge dummies between G1 and G2
    "dve_relu01_first": False,  # relu(0,1) before relu(1,0) on DVE
    # mid-stream load order (xsp groups always follow); gt/wsp/tail ride
    # AFTER xt9 so GEMM1's last k-step lands ~0.8us earlier
    "load_order": "a0 x0 x1 x2 a3 x3 x4 bs x5 x6 a7 x7 x8 x9 tl gt ws",
}


def _dft_constants():
    """F [2688, 2688] (rfft, ortho, dead rows dropped) and G [384, 384]
    (irfft, ortho, dead cols dropped)."""
    if "F" in _cache:
        return _cache["F"], _cache["G"]
    Fc = np.fft.rfft(np.eye(H), axis=0, norm="ortho")       # [1345, 2688]
    F = np.concatenate([Fc.real, Fc.imag[1:FREQ_IN - 1]], axis=0)
    F = np.ascontiguousarray(F, dtype=np.float32)           # [2688, 2688]
    G_re = np.fft.irfft(np.eye(FREQ_OUT), n=OUT_H, axis=0, norm="ortho")
    G_im = np.fft.irfft(1j * np.eye(FREQ_OUT), n=OUT_H, axis=0, norm="ortho")
    G = np.concatenate([G_re, G_im[:, 1:FREQ_OUT - 1]], axis=1)
    G = np.ascontiguousarray(G, dtype=np.float32)           # [384, 384]
    _cache["F"] = F
    _cache["G"] = G
    return F, G


def _spec_keep_idx():
    keep_f = list(range(FREQ_IN)) + [FREQ_IN + k for k in range(1, FREQ_IN - 1)]
    keep_o = list(range(FREQ_OUT)) + [FREQ_OUT + k for k in range(1, FREQ_OUT - 1)]
    return np.array(keep_f), np.array(keep_o)


def _build_main():
    key = ("main", repr(sorted(CFG.items())))
    if key in _cache:
        return _cache[key]
    nc = bacc.Bacc("TRN2", target_bir_lowering=False, debug=False,
                   num_devices=N_CORES)
    at = nc.dram_tensor("at", [128, KD_DR * 2 * MO], F8E4,
                        kind="ExternalInput").ap()
    xt = nc.dram_tensor("xt", [128, KD_DR * 2 * NCOL], F8E4,
                        kind="ExternalInput").ap()
    # packed DR tail: att [64, 768] and xtt [64, 2048] merged in one tensor
    tt = nc.dram_tensor("tt", [64, 2 * MO + 2 * NCOL], F8E4,
                        kind="ExternalInput").ap()
    gt = nc.dram_tensor("gt", [128, MT * MO], F16, kind="ExternalInput").ap()
    bspec = nc.dram_tensor("bspec", [128, MT], F32, kind="ExternalInput").ap()
    wsp = nc.dram_tensor("wsp", [KHB, 2 * C], F8E4, kind="ExternalInput").ap()
    xsp = nc.dram_tensor("xsp", [KHB, 2 * NSP], F8E4,
                         kind="ExternalInput").ap()
    # last spatial group (b=1, m2=2) w-split into [12w | 4w] pieces so only
    # a small fadd trails the final input transfer
    xspL = nc.dram_tensor("xspL", [KHB, 2 * 2048], F8E4,
                          kind="ExternalInput").ap()
    out_d = nc.dram_tensor("out", [128, MT * NCOL], F16,
                           kind="ExternalOutput").ap()

    with tile.TileContext(nc) as tc:
        with tc.tile_pool(name="const", bufs=1) as cst, \
             tc.tile_pool(name="atp", bufs=1) as atp, \
             tc.tile_pool(name="xtp", bufs=1) as xtp, \
             tc.tile_pool(name="xspp", bufs=1) as xspp, \
             tc.tile_pool(name="relu", bufs=1) as rlp, \
             tc.tile_pool(name="outp", bufs=1) as outp, \
             tc.tile_pool(name="ps", bufs=1, space="PSUM") as psp, \
             tc.tile_pool(name="psw", bufs=1, space="PSUM") as psw:

            # ---------------- SBUF tiles ----------------
            AGRP = CFG.get("agrp", [(0, 3), (3, 7), (7, KD_DR)])
            at_g = {}
            for g0, g1 in AGRP:
                at_g[g0] = atp.tile([128, (g1 - g0) * 2 * MO], F8E4,
                                    tag=f"at{g0}", name=f"at{g0}")
            xt_t = [xtp.tile([128, 2 * NCOL], F8E4, tag=f"xt{s}",
                             name=f"xt{s}") for s in range(KD_DR)]
            tt_sb = xtp.tile([64, 2 * MO + 2 * NCOL], F8E4, tag="tt",
                             name="tt")
            gt_sb = cst.tile([128, MT * MO], F16, tag="gt", name="gt")
            bspec_sb = cst.tile([128, MT], F32, tag="bspec", name="bspec")
            wsp_sb = cst.tile([KHB, 2 * C], F8E4, tag="wsp", name="wsp")
            xsp_g = {(b, m2): xspp.tile([KHB, 2 * 2048], F8E4,
                                        tag=f"xsp{b}{m2}", name=f"xsp{b}{m2}")
                     for b in range(B) for m2 in range(MT)
                     if (b, m2) != (1, 2)}
            xspA = xspp.tile([KHB, 2 * 1536], F8E4, tag="xspA", name="xspA")
            xspB = xspp.tile([KHB, 2 * 512], F8E4, tag="xspB", name="xspB")
            # one tile per independently produced/consumed chunk -- the tile
            # framework tracks dependencies at TILE granularity, so shared
            # tiles would serialize unrelated producers/consumers
            relu_h = {(m, n): rlp.tile([128, 512], F16, tag=f"relu{m}{n}",
                                       name=f"relu{m}{n}")
                      for m in range(MT) for n in range(NT)}
            spec_sb = {(m2, n): rlp.tile([128, 512], F16, tag=f"sc{m2}{n}",
                                         name=f"sc{m2}{n}")
                       for m2 in range(MT) for n in range(NT)}
            # output SBUF grouped by store: A = all n0 chunks (ready ~same
            # time), B = (0,1)+(1,1), C = (2,1) alone -- three stores instead
            # of six halves the HWDGE generation serialization at the tail
            out_A = outp.tile([128, 1536], F16, tag="oA", name="oA")
            out_B = outp.tile([128, 1024], F16, tag="oB", name="oB")
            out_C = outp.tile([128, 512], F16, tag="oC", name="oC")

            def out_slice(m2, n):
                if n == 0:
                    return out_A[:, 512 * m2:512 * (m2 + 1)]
                if m2 == 2:
                    return out_C[:]
                return out_B[:, 512 * m2:512 * (m2 + 1)]

            def at_s(s, m):
                """DR stationary [*, 2, 128] for DR step s, m-tile m."""
                if s == KD_DR:
                    v = tt_sb[:, 0:2 * MO].rearrange("p (i m) -> p i m", i=2)
                else:
                    g0 = max(g for g, _ in AGRP if g <= s)
                    off = (s - g0) * 2 * MO
                    v = at_g[g0][:, off:off + 2 * MO].rearrange(
                        "p (i m) -> p i m", i=2)
                return v[:, :, 128 * m:128 * (m + 1)]

            def xt_s(s, n):
                if s == KD_DR:
                    v = tt_sb[:, 2 * MO:].rearrange("p (i n) -> p i n", i=2)
                else:
                    v = xt_t[s][:].rearrange("p (i n) -> p i n", i=2)
                return v[:, :, 512 * n:512 * (n + 1)]

            wsp_v = wsp_sb[:].rearrange("p (i m) -> p i m", i=2)
            gt_km = lambda k, m2: gt_sb[:, k * MO + 128 * m2:
                                        k * MO + 128 * (m2 + 1)]

            # ---------------- DMA emission (sync queue, in order) --------
            def load_at(g0):
                g1 = dict(AGRP)[g0]
                nc.sync.dma_start(at_g[g0][:],
                                  at[:, g0 * 2 * MO:g1 * 2 * MO])

            def load_xt(s):
                nc.sync.dma_start(xt_t[s][:],
                                  xt[:, s * 2 * NCOL:(s + 1) * 2 * NCOL])

            def load_tail():
                nc.sync.dma_start(tt_sb[:], tt[:])

            def load_xsp(b, m2):
                if (b, m2) == (1, 2):
                    srcL = xspL.rearrange("p (i n) -> p i n", i=2)
                    nc.sync.dma_start(
                        xspA[:].rearrange("p (i n) -> p i n", i=2),
                        srcL[:, :, 0:1536])
                    nc.sync.dma_start(
                        xspB[:].rearrange("p (i n) -> p i n", i=2),
                        srcL[:, :, 1536:2048])
                    return
                src = xsp.rearrange("p (i n) -> p i n", i=2)[
                    :, :, b * (OUT_H * WS) + m2 * 2048:
                    b * (OUT_H * WS) + (m2 + 1) * 2048]
                dst = xsp_g[(b, m2)][:].rearrange("p (i n) -> p i n", i=2)
                nc.sync.dma_start(dst, src)

            # order tuned so the PE (starting at xt0+900ns sem prop) never
            # starves; small consts hide mid-stream behind big transfers; the
            # last input (xsp b1 m2=2) has the shortest dependent chain
            loaders = {
                # a0/a3/a7 historically named; they mean AGRP groups 0/1/2
                "a0": lambda: load_at(AGRP[0][0]),
                "a3": lambda: load_at(AGRP[1][0]),
                "a7": lambda: load_at(AGRP[2][0]) if len(AGRP) > 2 else None,
                "tl": load_tail,
                "gt": lambda: nc.sync.dma_start(gt_sb[:], gt[:]),
                "bs": lambda: nc.sync.dma_start(bspec_sb[:], bspec[:]),
                "ws": lambda: nc.sync.dma_start(wsp_sb[:], wsp[:]),
            }
            for tok in CFG["load_order"].split():
                if tok.startswith("x"):
                    load_xt(int(tok[1:]))
                else:
                    loaders[tok]()
            for b, m2 in CFG.get("xsp_order",
                                 [(0, 0), (0, 1), (0, 2),
                                  (1, 0), (1, 1), (1, 2)]):
                load_xsp(b, m2)

            # ---------------- compute ----------------
            ps1 = {(m, n): psp.tile([128, 512], F32, tag=f"g1m{m}n{n}",
                                    name=f"g1m{m}n{n}")
                   for m in range(MT) for n in range(NT)}

            def g1_step(s):
                # s == KD_DR is the [64, 2, *] packed tail step
                for n in range(NT):
                    for m in range(MT):
                        nc.tensor.matmul(ps1[(m, n)][:], at_s(s, m),
                                         xt_s(s, n), start=(s == 0),
                                         stop=(s == KD_DR), perf_mode=DR)

            sp_ps = {}

            def sp_chunk(b, m2):
                # transposed spatial conv: stationary = x slices, moving = w.
                # 16 tiny DR matmuls land [h'-block, (w,co)] directly in the
                # spectral output layout.
                j = b * MT + m2
                ps = psw.tile([128, 512], F32, tag=f"spp{j % 2}",
                              name=f"spp{j}")
                sp_ps[(b, m2)] = ps
                xv = xsp_g[(b, m2)][:].rearrange("p (i n) -> p i n", i=2)
                for w in range(WS):
                    nc.tensor.matmul(ps[:, 32 * w:32 * (w + 1)],
                                     xv[:, :, w::WS], wsp_v,
                                     start=True, stop=True, perf_mode=DR)

            def sp_piece(which):
                # (1,2) split: separate psum tiles so piece A's add never
                # waits piece B's (later) DMA
                if which == 0:
                    ps = psw.tile([128, 384], F32, tag="spp1", name="sppA")
                    xv = xspA[:].rearrange("p (i n) -> p i n", i=2)
                    nw = 12
                else:
                    ps = psp.tile([128, 128], F32, tag="g1m0n0", name="sppB")
                    xv = xspB[:].rearrange("p (i n) -> p i n", i=2)
                    nw = 4
                sp_ps[("L", which)] = ps
                for w in range(nw):
                    nc.tensor.matmul(ps[:, 32 * w:32 * (w + 1)],
                                     xv[:, :, w::nw], wsp_v,
                                     start=True, stop=True, perf_mode=DR)

            # n-split relus let g2(n0) start as soon as the last k-step's
            # (m, n0) groups close; ACT/DVE checkerboard
            RELU_ACT = {(0, 0), (2, 0), (1, 1)}

            def relu_m(m, n):
                # relu1 scaled x64 (G absorbs /64); only ACT/DVE read PSUM
                if (m, n) in RELU_ACT:
                    nc.scalar.activation(relu_h[(m, n)][:], ps1[(m, n)][:],
                                         RELU, bias=bspec_sb[:, m:m + 1])
                else:
                    nc.vector.tensor_scalar(relu_h[(m, n)][:], ps1[(m, n)][:],
                                            bspec_sb[:, m:m + 1], 0.0,
                                            AluOpType.add, AluOpType.max)

            ps2 = {}

            def g2_n(n):
                for m2 in range(MT):
                    ps2[(m2, n)] = psp.tile([128, 512], F32,
                                            tag=f"g1m{m2}n{n}",
                                            name=f"g2m{m2}n{n}")
                if CFG.get("g2_m_major"):
                    # m-major: each m2 psum group closes ASAP, feeding the
                    # ACT spec-copy -> DVE fadd staircase earlier
                    for m2 in range(MT):
                        for k in range(MT):
                            nc.tensor.matmul(ps2[(m2, n)][:], gt_km(k, m2),
                                             relu_h[(k, n)][:],
                                             start=(k == 0),
                                             stop=(k == MT - 1))
                else:
                    for k in range(MT):
                        for m2 in range(MT):
                            nc.tensor.matmul(ps2[(m2, n)][:], gt_km(k, m2),
                                             relu_h[(k, n)][:],
                                             start=(k == 0),
                                             stop=(k == MT - 1))

            def copy_spec(m2, n):
                # spec psum -> f16 sbuf on the otherwise-idle ACT, well
                # before the spatial psum arrives -- keeps the tail chain to
                # a single DVE op per chunk
                nc.scalar.activation(spec_sb[(m2, n)][:], ps2[(m2, n)][:],
                                     mybir.ActivationFunctionType.Copy)

            def fadd(m2, n):
                # out = relu(spatial psum) + spec(sbuf) in ONE DVE op; both
                # branches are x32 scaled (the host divides the output by 32)
                nc.vector.scalar_tensor_tensor(
                    out_slice(m2, n), sp_ps[(n, m2)][:], 0.0,
                    spec_sb[(m2, n)][:], AluOpType.max, AluOpType.add)

            # out DRAM columns are completion-ordered: [n0m0 n0m1 n0m2
            # n1m0 n1m1 n1m2]; the host unshard accounts for this
            def store_A():
                nc.sync.dma_start(out_d[:, 0:1536], out_A[:])

            def store_B():
                nc.sync.dma_start(out_d[:, 1536:2560], out_B[:])

            def store_C():
                nc.sync.dma_start(out_d[:, 2560:3072], out_C[:])

            def pe_warm(i):
                # tiny dummy matmul bridging the relu-latency gap between
                # GEMM1's last k-step and GEMM2, so the PE p-state (and with
                # it GEMM2's 2.4GHz rate) survives the wait. Reads the LAST
                # xt tile so the scheduler cannot hoist it earlier.
                ps = psw.tile([1, 256], F32, tag="spp0", name=f"warm{i}")
                nc.tensor.matmul(ps[:], xt_t[KD_DR - 1][:, 0:1],
                                 xt_t[KD_DR - 1][:, 0:256],
                                 start=True, stop=True)

            # ---- PE order ----
            for s in range(KD_DR + 1):
                g1_step(s)
            relu_m(0, 0)
            if CFG["dve_relu01_first"]:
                relu_m(0, 1)
                relu_m(2, 0)
                relu_m(1, 0)
            else:
                relu_m(1, 0)
                relu_m(2, 0)
                relu_m(0, 1)
            relu_m(1, 1)
            relu_m(2, 1)
            for i in range(CFG["warms"]):
                pe_warm(i)
            g2_n(0)
            sp_chunk(0, 0)
            sp_chunk(0, 1)
            sp_chunk(0, 2)
            copy_spec(0, 0)
            copy_spec(1, 0)
            copy_spec(2, 0)
            g2_n(1)
            if CFG.get("sc21_first"):
                copy_spec(2, 1)
                copy_spec(0, 1)
                copy_spec(1, 1)
            else:
                copy_spec(0, 1)
                copy_spec(1, 1)
                copy_spec(2, 1)
            fadd(0, 0)
            fadd(1, 0)
            fadd(2, 0)
            store_A()
            sp_chunk(1, 0)
            fadd(0, 1)
            sp_chunk(1, 1)
            fadd(1, 1)
            store_B()
            sp_piece(0)
            nc.vector.scalar_tensor_tensor(
                out_C[:, 0:384], sp_ps[("L", 0)][:], 0.0,
                spec_sb[(2, 1)][:, 0:384], AluOpType.max, AluOpType.add)
            sp_piece(1)
            nc.vector.scalar_tensor_tensor(
                out_C[:, 384:512], sp_ps[("L", 1)][:], 0.0,
                spec_sb[(2, 1)][:, 384:512], AluOpType.max, AluOpType.add)
            store_C()

    nc.compile()
    _cache["main"] = nc
    return nc


def _host_prep(x, w_spatial, b_spatial, w_spectral, b_spectral):
    """Shared (weight) swizzles."""
    F, G = _dft_constants()
    keep_f, keep_o = _spec_keep_idx()

    A = w_spectral[keep_o][:, keep_f] @ F                    # [384, 2688]
    arrA = np.ascontiguousarray((A * AT_SCALE).T)            # [2688, 384]
    hcut = KD_DR * 256
    at_np = np.ascontiguousarray(
        arrA[:hcut].reshape(KD_DR * 2, 128, MO).transpose(1, 0, 2)
        .reshape(128, KD_DR * 2 * MO)).astype(E4M3)
    att_np = (arrA[hcut:].reshape(2, 64, MO).transpose(1, 0, 2)
              .reshape(64, 2 * MO)).astype(E4M3)
    # gt absorbs both the relu1 x64 scale and the x32 output scale (the
    # device emits 32*(spatial+spectral); the host divides by 32)
    gt_np = np.ascontiguousarray(
        (G.T * (WSP_SCALE / AT_SCALE)).reshape(MT, 128, MO).transpose(1, 0, 2)
        .reshape(128, MT * MO)).astype(np.float16)
    bspec_np = np.ascontiguousarray(
        (b_spectral[keep_o] * AT_SCALE).reshape(MT, 128).T).astype(np.float32)
    wbase = (w_spatial[:, :, :, 0].transpose(1, 2, 0).reshape(C * 7, C)
             * WSP_SCALE)
    wsp_np = np.concatenate([
        wbase.reshape(2, KH, C).transpose(1, 0, 2).reshape(KH, 2 * C),
        np.concatenate([b_spatial * WSP_SCALE, np.zeros(C)])[None, :],
    ], axis=0).astype(E4M3)                                  # [113, 64]
    return at_np, att_np, gt_np, bspec_np, wsp_np


def kernel(x, w_spatial, b_spatial, w_spectral, b_spectral):
    x = np.ascontiguousarray(x, dtype=np.float32)
    w_spatial = np.asarray(w_spatial, dtype=np.float32)
    b_spatial = np.asarray(b_spatial, dtype=np.float32)
    w_spectral = np.asarray(w_spectral, dtype=np.float32)
    b_spectral = np.asarray(b_spectral, dtype=np.float32)

    at_np, att_np, gt_np, bspec_np, wsp_np = _host_prep(
        x, w_spatial, b_spatial, w_spectral, b_spectral)
    core_ids = list(range(N_CORES))
    hcut = KD_DR * 256

    in_maps = []
    for i in core_ids:
        xs = x[:, :, :, WS * i:WS * (i + 1)]                 # [B, C, H, WS]
        arr = xs.transpose(2, 0, 3, 1).reshape(H, NCOL)      # [H, (b,w,c)]
        xt_np = np.ascontiguousarray(
            arr[:hcut].reshape(KD_DR * 2, 128, NCOL).transpose(1, 0, 2)
            .reshape(128, KD_DR * 2 * NCOL)).astype(E4M3)
        xtt_np = (arr[hcut:].reshape(2, 64, NCOL).transpose(1, 0, 2)
                  .reshape(64, 2 * NCOL)).astype(E4M3)
        tt_np = np.ascontiguousarray(
            np.concatenate([att_np, xtt_np], axis=1))        # [64, 2816]
        spbase = (xs.reshape(B, C, OUT_H, 7, WS).transpose(1, 3, 0, 2, 4)
                  .reshape(C * 7, NSP))                      # [(c,t),(b,h',w)]
        xsp_np = np.concatenate([
            spbase.reshape(2, KH, NSP).transpose(1, 0, 2).reshape(KH, 2 * NSP),
            np.concatenate([np.ones(NSP, np.float32),
                            np.zeros(NSP, np.float32)])[None, :],
        ], axis=0).astype(E4M3)                              # [113, 2*NSP]
        # last group (b=1, m2=2) w-split [12w | 4w], cols (w-block, h', w)
        grp = spbase[:, 5 * 2048:6 * 2048].reshape(C * 7, 128, WS)
        cat = np.concatenate([grp[:, :, :12].reshape(C * 7, 1536),
                              grp[:, :, 12:].reshape(C * 7, 512)], axis=1)
        xspL_np = np.concatenate([
            cat.reshape(2, KH, 2048).transpose(1, 0, 2).reshape(KH, 4096),
            np.concatenate([np.ones(2048, np.float32),
                            np.zeros(2048, np.float32)])[None, :],
        ], axis=0).astype(E4M3)                              # [113, 4096]
        in_maps.append({"at": at_np, "xt": xt_np, "tt": tt_np,
                        "gt": gt_np, "bspec": bspec_np, "wsp": wsp_np,
                        "xsp": xsp_np, "xspL": xspL_np})

    nc = _build_main()
    kw = {}
    if bool(int(os.environ.get("KERNEL_TRACE", "0"))):
        d = os.environ.get("KERNEL_TRACE_DIR", "/tmp/ktrace") + "/main"
        os.makedirs(d, exist_ok=True)
        kw = dict(trace=True, tmpdir=d)
    res = run_bass_kernel_spmd(nc, in_maps, core_ids, **kw)
    global LAST_EXEC_NS
    LAST_EXEC_NS = res.exec_time_ns

    # ---- host: unshard + undo the x32 device scale; device columns are
    # completion-ordered [n, m2] ----
    out = np.empty((B, C, OUT_H, W), np.float32)
    for i in core_ids:
        o = (res.results[i]["out"].astype(np.float32)
             .reshape(128, NT, MT, WS, C).transpose(1, 4, 2, 0, 3)
             .reshape(B, C, OUT_H, WS))
        out[:, :, :, WS * i:WS * (i + 1)] = o * (1.0 / WSP_SCALE)
    return out


# revision 120
# speedup vs baseline: 1.0142x; 1.0075x over previous
"""Trainium2 Bass kernel for the FFTBlock problem (B=2, C=32, H=2688, W=128).

Math (reference):
  spatial  = relu(conv7x1_s7(x) + b_spatial)                        [B,C,384,W]
  spectral = irfft(relu(w_spectral @ rfft_concat(x) + b_spectral))  per (b,c,w)
  out = spatial + spectral

rfft/irfft along H are linear, so with F the real-ified rfft matrix and G the
irfft matrix (dead rows/cols dropped):
  spectral_col = G @ relu(A @ x_col + b),   A = w_spectral @ F  [384, 2688]

Device plan (W sharded 8 x 16 columns, one launch per core):
  GEMM1  conv[384, 1024] = A @ x_cols: 10 DoubleRow steps (K=256, both
         operands e4m3, x64 A scale) + one [64, 2, *] packed DR tail step.
  relu   (ACT/DVE, bias, n-split so GEMM2 starts per-half) -> f16 (x64;
         G absorbs the scale)
  GEMM2  spec[384, 1024] = (G*32/64) @ relu  (f16, 3 k-tiles); the psum is
         copied to f16 SBUF on the otherwise-idle ACT.
  spatial conv TRANSPOSED: stationary = x chunks [(c,t)=224+bias row,
         h'-block], moving = w_spatial (e4m3 x32, bias folded as an extra
         contraction row), DoubleRow -> psum [h', (b,w,co)] -- same layout as
         spec, so the add happens on-device and only ONE output is stored.
  out = relu(spatial psum) + spec(sbuf) in ONE DVE scalar_tensor_tensor per
         chunk, f16 x32-scaled (host divides by 32); three grouped stores.

Column order everywhere is (b, w, c) so the spatial conv's 32-channel output
blocks are contiguous in the spectral column space. All DRAM layouts are
pre-swizzled on host to partition-major so every DMA moves >=512B runs; the
load order is tuned so the PE never starves and the last input (xsp b1 m2=2)
has the shortest dependent chain.
"""

import os

import numpy as np
import ml_dtypes

import concourse.bacc as bacc
import concourse.mybir as mybir
import concourse.tile as tile
from concourse.bass_utils import run_bass_kernel_spmd
from concourse.alu_op_type import AluOpType

N_CORES = 8
B, C, H, W = 2, 32, 2688, 128
FREQ_IN = H // 2 + 1            # 1345
OUT_H = 384
FREQ_OUT = OUT_H // 2 + 1       # 193
MO = 2 * FREQ_OUT - 2           # 384 usable conv channels
WS = W // N_CORES               # 16 width columns per core
NCOL = B * WS * C               # 1024 spectral columns per core, (b, w, c)
NSP = B * OUT_H * WS            # 12288 spatial cols (b, h', w)
OLDK = H // 128                 # 21 k-tiles of 128
KD_DR = 10                      # full DoubleRow steps (old-k 0..2*KD_DR-1)
# the odd tail tile (old-k 20) runs as a [64, 2, *] DoubleRow step
MT = 3                          # 128-row m-tiles (G1 out / G2 out)
NT = 2                          # 512-col n halves; n == b
KH = 112                        # (c,t) DR half-pairs for spatial
KHB = KH + 1                    # +1 bias row

AT_SCALE = 64.0                 # fp8 range helper for A = w_spec @ F
WSP_SCALE = 32.0                # fp8 range helper for the tiny spatial weights

F32 = mybir.dt.float32
F16 = mybir.dt.float16
F8E4 = mybir.dt.float8e4
F8E3 = mybir.dt.float8e3
RELU = mybir.ActivationFunctionType.Relu
DR = mybir.MatmulPerfMode.DoubleRow
E4M3 = ml_dtypes.float8_e4m3
E3M4 = ml_dtypes.float8_e3m4

_cache = {}
LAST_EXEC_NS = None

# scheduling knobs (tuned against TimelineSim; see tuner.py)
CFG = {
    "warms": 0,           # p-state bridge dummies between G1 and G2
    "dve_relu01_first": False,  # relu(0,1) before relu(1,0) on DVE
    # mid-stream load order (xsp groups always follow); gt/wsp/tail ride
    # AFTER xt9 so GEMM1's last k-step lands ~0.8us earlier
    "load_order": "a0 x0 x1 x2 a3 x3 x4 bs x5 x6 a7 x7 x8 x9 tl gt ws",
    # w-split point of the last spatial group (piece A size in w's)
    "aw": 8,
}


def _dft_constants():
    """F [2688, 2688] (rfft, ortho, dead rows dropped) and G [384, 384]
    (irfft, ortho, dead cols dropped)."""
    if "F" in _cache:
        return _cache["F"], _cache["G"]
    Fc = np.fft.rfft(np.eye(H), axis=0, norm="ortho")       # [1345, 2688]
    F = np.concatenate([Fc.real, Fc.imag[1:FREQ_IN - 1]], axis=0)
    F = np.ascontiguousarray(F, dtype=np.float32)           # [2688, 2688]
    G_re = np.fft.irfft(np.eye(FREQ_OUT), n=OUT_H, axis=0, norm="ortho")
    G_im = np.fft.irfft(1j * np.eye(FREQ_OUT), n=OUT_H, axis=0, norm="ortho")
    G = np.concatenate([G_re, G_im[:, 1:FREQ_OUT - 1]], axis=1)
    G = np.ascontiguousarray(G, dtype=np.float32)           # [384, 384]
    _cache["F"] = F
    _cache["G"] = G
    return F, G


def _spec_keep_idx():
    keep_f = list(range(FREQ_IN)) + [FREQ_IN + k for k in range(1, FREQ_IN - 1)]
    keep_o = list(range(FREQ_OUT)) + [FREQ_OUT + k for k in range(1, FREQ_OUT - 1)]
    return np.array(keep_f), np.array(keep_o)


def _build_main():
    key = ("main", repr(sorted(CFG.items())))
    if key in _cache:
        return _cache[key]
    nc = bacc.Bacc("TRN2", target_bir_lowering=False, debug=False,
                   num_devices=N_CORES)
    at = nc.dram_tensor("at", [128, KD_DR * 2 * MO], F8E4,
                        kind="ExternalInput").ap()
    xt = nc.dram_tensor("xt", [128, KD_DR * 2 * NCOL], F8E4,
                        kind="ExternalInput").ap()
    # packed DR tail: att [64, 768] and xtt [64, 2048] merged in one tensor
    tt = nc.dram_tensor("tt", [64, 2 * MO + 2 * NCOL], F8E4,
                        kind="ExternalInput").ap()
    gt = nc.dram_tensor("gt", [128, MT * MO], F16, kind="ExternalInput").ap()
    bspec = nc.dram_tensor("bspec", [128, MT], F32, kind="ExternalInput").ap()
    wsp = nc.dram_tensor("wsp", [KHB, 2 * C], F8E4, kind="ExternalInput").ap()
    xsp = nc.dram_tensor("xsp", [KHB, 2 * NSP], F8E4,
                         kind="ExternalInput").ap()
    # last spatial group (b=1, m2=2) w-split into [12w | 4w] pieces so only
    # a small fadd trails the final input transfer
    xspL = nc.dram_tensor("xspL", [KHB, 2 * 2048], F8E4,
                          kind="ExternalInput").ap()
    out_d = nc.dram_tensor("out", [128, MT * NCOL], F16,
                           kind="ExternalOutput").ap()

    with tile.TileContext(nc) as tc:
        with tc.tile_pool(name="const", bufs=1) as cst, \
             tc.tile_pool(name="atp", bufs=1) as atp, \
             tc.tile_pool(name="xtp", bufs=1) as xtp, \
             tc.tile_pool(name="xspp", bufs=1) as xspp, \
             tc.tile_pool(name="relu", bufs=1) as rlp, \
             tc.tile_pool(name="outp", bufs=1) as outp, \
             tc.tile_pool(name="ps", bufs=1, space="PSUM") as psp, \
             tc.tile_pool(name="psw", bufs=1, space="PSUM") as psw:

            # ---------------- SBUF tiles ----------------
            AGRP = CFG.get("agrp", [(0, 3), (3, 7), (7, KD_DR)])
            at_g = {}
            for g0, g1 in AGRP:
                at_g[g0] = atp.tile([128, (g1 - g0) * 2 * MO], F8E4,
                                    tag=f"at{g0}", name=f"at{g0}")
            xt_t = [xtp.tile([128, 2 * NCOL], F8E4, tag=f"xt{s}",
                             name=f"xt{s}") for s in range(KD_DR)]
            tt_sb = xtp.tile([64, 2 * MO + 2 * NCOL], F8E4, tag="tt",
                             name="tt")
            gt_sb = cst.tile([128, MT * MO], F16, tag="gt", name="gt")
            bspec_sb = cst.tile([128, MT], F32, tag="bspec", name="bspec")
            wsp_sb = cst.tile([KHB, 2 * C], F8E4, tag="wsp", name="wsp")
            xsp_g = {(b, m2): xspp.tile([KHB, 2 * 2048], F8E4,
                                        tag=f"xsp{b}{m2}", name=f"xsp{b}{m2}")
                     for b in range(B) for m2 in range(MT)
                     if (b, m2) != (1, 2)}
            # split point: piece A = first AW w's, piece B = the rest
            AW = CFG.get("aw", 8)
            xspA = xspp.tile([KHB, 2 * 128 * AW], F8E4, tag="xspA",
                             name="xspA")
            xspB = xspp.tile([KHB, 2 * 128 * (WS - AW)], F8E4, tag="xspB",
                             name="xspB")
            # one tile per independently produced/consumed chunk -- the tile
            # framework tracks dependencies at TILE granularity, so shared
            # tiles would serialize unrelated producers/consumers
            relu_h = {(m, n): rlp.tile([128, 512], F16, tag=f"relu{m}{n}",
                                       name=f"relu{m}{n}")
                      for m in range(MT) for n in range(NT)}
            spec_sb = {(m2, n): rlp.tile([128, 512], F16, tag=f"sc{m2}{n}",
                                         name=f"sc{m2}{n}")
                       for m2 in range(MT) for n in range(NT)}
            # output SBUF grouped by store: A = all n0 chunks (ready ~same
            # time), B = (0,1)+(1,1), C = (2,1) alone -- three stores instead
            # of six halves the HWDGE generation serialization at the tail
            out_A = outp.tile([128, 1536], F16, tag="oA", name="oA")
            out_B = outp.tile([128, 1024], F16, tag="oB", name="oB")
            out_C = outp.tile([128, 512], F16, tag="oC", name="oC")

            def out_slice(m2, n):
                if n == 0:
                    return out_A[:, 512 * m2:512 * (m2 + 1)]
                if m2 == 2:
                    return out_C[:]
                return out_B[:, 512 * m2:512 * (m2 + 1)]

            def at_s(s, m):
                """DR stationary [*, 2, 128] for DR step s, m-tile m."""
                if s == KD_DR:
                    v = tt_sb[:, 0:2 * MO].rearrange("p (i m) -> p i m", i=2)
                else:
                    g0 = max(g for g, _ in AGRP if g <= s)
                    off = (s - g0) * 2 * MO
                    v = at_g[g0][:, off:off + 2 * MO].rearrange(
                        "p (i m) -> p i m", i=2)
                return v[:, :, 128 * m:128 * (m + 1)]

            def xt_s(s, n):
                if s == KD_DR:
                    v = tt_sb[:, 2 * MO:].rearrange("p (i n) -> p i n", i=2)
                else:
                    v = xt_t[s][:].rearrange("p (i n) -> p i n", i=2)
                return v[:, :, 512 * n:512 * (n + 1)]

            wsp_v = wsp_sb[:].rearrange("p (i m) -> p i m", i=2)
            gt_km = lambda k, m2: gt_sb[:, k * MO + 128 * m2:
                                        k * MO + 128 * (m2 + 1)]

            # ---------------- DMA emission (sync queue, in order) --------
            def load_at(g0):
                g1 = dict(AGRP)[g0]
                nc.sync.dma_start(at_g[g0][:],
                                  at[:, g0 * 2 * MO:g1 * 2 * MO])

            def load_xt(s):
                nc.sync.dma_start(xt_t[s][:],
                                  xt[:, s * 2 * NCOL:(s + 1) * 2 * NCOL])

            def load_tail():
                nc.sync.dma_start(tt_sb[:], tt[:])

            def load_xsp(b, m2):
                if (b, m2) == (1, 2):
                    srcL = xspL.rearrange("p (i n) -> p i n", i=2)
                    cut = 128 * AW
                    nc.sync.dma_start(
                        xspA[:].rearrange("p (i n) -> p i n", i=2),
                        srcL[:, :, 0:cut])
                    nc.sync.dma_start(
                        xspB[:].rearrange("p (i n) -> p i n", i=2),
                        srcL[:, :, cut:2048])
                    return
                src = xsp.rearrange("p (i n) -> p i n", i=2)[
                    :, :, b * (OUT_H * WS) + m2 * 2048:
                    b * (OUT_H * WS) + (m2 + 1) * 2048]
                dst = xsp_g[(b, m2)][:].rearrange("p (i n) -> p i n", i=2)
                nc.sync.dma_start(dst, src)

            # order tuned so the PE (starting at xt0+900ns sem prop) never
            # starves; small consts hide mid-stream behind big transfers; the
            # last input (xsp b1 m2=2) has the shortest dependent chain
            loaders = {
                # a0/a3/a7 historically named; they mean AGRP groups 0/1/2
                "a0": lambda: load_at(AGRP[0][0]),
                "a3": lambda: load_at(AGRP[1][0]),
                "a7": lambda: load_at(AGRP[2][0]) if len(AGRP) > 2 else None,
                "tl": load_tail,
                "gt": lambda: nc.sync.dma_start(gt_sb[:], gt[:]),
                "bs": lambda: nc.sync.dma_start(bspec_sb[:], bspec[:]),
                "ws": lambda: nc.sync.dma_start(wsp_sb[:], wsp[:]),
            }
            for tok in CFG["load_order"].split():
                if tok.startswith("x"):
                    load_xt(int(tok[1:]))
                else:
                    loaders[tok]()
            for b, m2 in CFG.get("xsp_order",
                                 [(0, 0), (0, 1), (0, 2),
                                  (1, 0), (1, 1), (1, 2)]):
                load_xsp(b, m2)

            # ---------------- compute ----------------
            ps1 = {(m, n): psp.tile([128, 512], F32, tag=f"g1m{m}n{n}",
                                    name=f"g1m{m}n{n}")
                   for m in range(MT) for n in range(NT)}

            def g1_step(s):
                # s == KD_DR is the [64, 2, *] packed tail step
                for n in range(NT):
                    for m in range(MT):
                        nc.tensor.matmul(ps1[(m, n)][:], at_s(s, m),
                                         xt_s(s, n), start=(s == 0),
                                         stop=(s == KD_DR), perf_mode=DR)

            sp_ps = {}

            def sp_chunk(b, m2):
                # transposed spatial conv: stationary = x slices, moving = w.
                # 16 tiny DR matmuls land [h'-block, (w,co)] directly in the
                # spectral output layout.
                j = b * MT + m2
                ps = psw.tile([128, 512], F32, tag=f"spp{j % 2}",
                              name=f"spp{j}")
                sp_ps[(b, m2)] = ps
                xv = xsp_g[(b, m2)][:].rearrange("p (i n) -> p i n", i=2)
                for w in range(WS):
                    nc.tensor.matmul(ps[:, 32 * w:32 * (w + 1)],
                                     xv[:, :, w::WS], wsp_v,
                                     start=True, stop=True, perf_mode=DR)

            def sp_piece(which):
                # (1,2) split: separate psum tiles so piece A's add never
                # waits piece B's (later) DMA
                if which == 0:
                    nw = AW
                    ps = psw.tile([128, 32 * nw], F32, tag="spp1",
                                  name="sppA")
                    xv = xspA[:].rearrange("p (i n) -> p i n", i=2)
                else:
                    nw = WS - AW
                    ps = psp.tile([128, 32 * nw], F32, tag="g1m0n0",
                                  name="sppB")
                    xv = xspB[:].rearrange("p (i n) -> p i n", i=2)
                sp_ps[("L", which)] = ps
                for w in range(nw):
                    nc.tensor.matmul(ps[:, 32 * w:32 * (w + 1)],
                                     xv[:, :, w::nw], wsp_v,
                                     start=True, stop=True, perf_mode=DR)

            # n-split relus let g2(n0) start as soon as the last k-step's
            # (m, n0) groups close; ACT/DVE checkerboard
            RELU_ACT = {(0, 0), (2, 0), (1, 1)}

            def relu_m(m, n):
                # relu1 scaled x64 (G absorbs /64); only ACT/DVE read PSUM
                if (m, n) in RELU_ACT:
                    nc.scalar.activation(relu_h[(m, n)][:], ps1[(m, n)][:],
                                         RELU, bias=bspec_sb[:, m:m + 1])
                else:
                    nc.vector.tensor_scalar(relu_h[(m, n)][:], ps1[(m, n)][:],
                                            bspec_sb[:, m:m + 1], 0.0,
                                            AluOpType.add, AluOpType.max)

            ps2 = {}

            def g2_n(n):
                for m2 in range(MT):
                    ps2[(m2, n)] = psp.tile([128, 512], F32,
                                            tag=f"g1m{m2}n{n}",
                                            name=f"g2m{m2}n{n}")
                if CFG.get("g2_m_major"):
                    # m-major: each m2 psum group closes ASAP, feeding the
                    # ACT spec-copy -> DVE fadd staircase earlier
                    for m2 in range(MT):
                        for k in range(MT):
                            nc.tensor.matmul(ps2[(m2, n)][:], gt_km(k, m2),
                                             relu_h[(k, n)][:],
                                             start=(k == 0),
                                             stop=(k == MT - 1))
                else:
                    for k in range(MT):
                        for m2 in range(MT):
                            nc.tensor.matmul(ps2[(m2, n)][:], gt_km(k, m2),
                                             relu_h[(k, n)][:],
                                             start=(k == 0),
                                             stop=(k == MT - 1))

            def copy_spec(m2, n):
                # spec psum -> f16 sbuf on the otherwise-idle ACT, well
                # before the spatial psum arrives -- keeps the tail chain to
                # a single DVE op per chunk
                nc.scalar.activation(spec_sb[(m2, n)][:], ps2[(m2, n)][:],
                                     mybir.ActivationFunctionType.Copy)

            def fadd(m2, n):
                # out = relu(spatial psum) + spec(sbuf) in ONE DVE op; both
                # branches are x32 scaled (the host divides the output by 32)
                nc.vector.scalar_tensor_tensor(
                    out_slice(m2, n), sp_ps[(n, m2)][:], 0.0,
                    spec_sb[(m2, n)][:], AluOpType.max, AluOpType.add)

            # out DRAM columns are completion-ordered: [n0m0 n0m1 n0m2
            # n1m0 n1m1 n1m2]; the host unshard accounts for this
            def store_A():
                nc.sync.dma_start(out_d[:, 0:1536], out_A[:])

            def store_B():
                nc.sync.dma_start(out_d[:, 1536:2560], out_B[:])

            def store_C():
                nc.sync.dma_start(out_d[:, 2560:3072], out_C[:])

            def pe_warm(i):
                # tiny dummy matmul bridging the relu-latency gap between
                # GEMM1's last k-step and GEMM2, so the PE p-state (and with
                # it GEMM2's 2.4GHz rate) survives the wait. Reads the LAST
                # xt tile so the scheduler cannot hoist it earlier.
                ps = psw.tile([1, 256], F32, tag="spp0", name=f"warm{i}")
                nc.tensor.matmul(ps[:], xt_t[KD_DR - 1][:, 0:1],
                                 xt_t[KD_DR - 1][:, 0:256],
                                 start=True, stop=True)

            # ---- PE order ----
            for s in range(KD_DR + 1):
                g1_step(s)
            relu_m(0, 0)
            if CFG["dve_relu01_first"]:
                relu_m(0, 1)
                relu_m(2, 0)
                relu_m(1, 0)
            else:
                relu_m(1, 0)
                relu_m(2, 0)
                relu_m(0, 1)
            relu_m(1, 1)
            relu_m(2, 1)
            for i in range(CFG["warms"]):
                pe_warm(i)
            g2_n(0)
            sp_chunk(0, 0)
            sp_chunk(0, 1)
            sp_chunk(0, 2)
            copy_spec(0, 0)
            copy_spec(1, 0)
            copy_spec(2, 0)
            g2_n(1)
            if CFG.get("sc21_first"):
                copy_spec(2, 1)
                copy_spec(0, 1)
                copy_spec(1, 1)
            else:
                copy_spec(0, 1)
                copy_spec(1, 1)
                copy_spec(2, 1)
            fadd(0, 0)
            fadd(1, 0)
            fadd(2, 0)
            store_A()
            sp_chunk(1, 0)
            fadd(0, 1)
            sp_chunk(1, 1)
            fadd(1, 1)
            store_B()
            acut = 32 * AW
            sp_piece(0)
            nc.vector.scalar_tensor_tensor(
                out_C[:, 0:acut], sp_ps[("L", 0)][:], 0.0,
                spec_sb[(2, 1)][:, 0:acut], AluOpType.max, AluOpType.add)
            sp_piece(1)
            nc.vector.scalar_tensor_tensor(
                out_C[:, acut:512], sp_ps[("L", 1)][:], 0.0,
                spec_sb[(2, 1)][:, acut:512], AluOpType.max, AluOpType.add)
            store_C()

    nc.compile()
    _cache["main"] = nc
    return nc


def _host_prep(x, w_spatial, b_spatial, w_spectral, b_spectral):
    """Shared (weight) swizzles."""
    F, G = _dft_constants()
    keep_f, keep_o = _spec_keep_idx()

    A = w_spectral[keep_o][:, keep_f] @ F                    # [384, 2688]
    arrA = np.ascontiguousarray((A * AT_SCALE).T)            # [2688, 384]
    hcut = KD_DR * 256
    at_np = np.ascontiguousarray(
        arrA[:hcut].reshape(KD_DR * 2, 128, MO).transpose(1, 0, 2)
        .reshape(128, KD_DR * 2 * MO)).astype(E4M3)
    att_np = (arrA[hcut:].reshape(2, 64, MO).transpose(1, 0, 2)
              .reshape(64, 2 * MO)).astype(E4M3)
    # gt absorbs both the relu1 x64 scale and the x32 output scale (the
    # device emits 32*(spatial+spectral); the host divides by 32)
    gt_np = np.ascontiguousarray(
        (G.T * (WSP_SCALE / AT_SCALE)).reshape(MT, 128, MO).transpose(1, 0, 2)
        .reshape(128, MT * MO)).astype(np.float16)
    bspec_np = np.ascontiguousarray(
        (b_spectral[keep_o] * AT_SCALE).reshape(MT, 128).T).astype(np.float32)
    wbase = (w_spatial[:, :, :, 0].transpose(1, 2, 0).reshape(C * 7, C)
             * WSP_SCALE)
    wsp_np = np.concatenate([
        wbase.reshape(2, KH, C).transpose(1, 0, 2).reshape(KH, 2 * C),
        np.concatenate([b_spatial * WSP_SCALE, np.zeros(C)])[None, :],
    ], axis=0).astype(E4M3)                                  # [113, 64]
    return at_np, att_np, gt_np, bspec_np, wsp_np


def kernel(x, w_spatial, b_spatial, w_spectral, b_spectral):
    x = np.ascontiguousarray(x, dtype=np.float32)
    w_spatial = np.asarray(w_spatial, dtype=np.float32)
    b_spatial = np.asarray(b_spatial, dtype=np.float32)
    w_spectral = np.asarray(w_spectral, dtype=np.float32)
    b_spectral = np.asarray(b_spectral, dtype=np.float32)

    at_np, att_np, gt_np, bspec_np, wsp_np = _host_prep(
        x, w_spatial, b_spatial, w_spectral, b_spectral)
    core_ids = list(range(N_CORES))
    hcut = KD_DR * 256

    in_maps = []
    for i in core_ids:
        xs = x[:, :, :, WS * i:WS * (i + 1)]                 # [B, C, H, WS]
        arr = xs.transpose(2, 0, 3, 1).reshape(H, NCOL)      # [H, (b,w,c)]
        xt_np = np.ascontiguousarray(
            arr[:hcut].reshape(KD_DR * 2, 128, NCOL).transpose(1, 0, 2)
            .reshape(128, KD_DR * 2 * NCOL)).astype(E4M3)
        xtt_np = (arr[hcut:].reshape(2, 64, NCOL).transpose(1, 0, 2)
                  .reshape(64, 2 * NCOL)).astype(E4M3)
        tt_np = np.ascontiguousarray(
            np.concatenate([att_np, xtt_np], axis=1))        # [64, 2816]
        spbase = (xs.reshape(B, C, OUT_H, 7, WS).transpose(1, 3, 0, 2, 4)
                  .reshape(C * 7, NSP))                      # [(c,t),(b,h',w)]
        xsp_np = np.concatenate([
            spbase.reshape(2, KH, NSP).transpose(1, 0, 2).reshape(KH, 2 * NSP),
            np.concatenate([np.ones(NSP, np.float32),
                            np.zeros(NSP, np.float32)])[None, :],
        ], axis=0).astype(E4M3)                              # [113, 2*NSP]
        # last group (b=1, m2=2) w-split [AW | WS-AW], cols (w-block, h', w)
        aw = CFG.get("aw", 8)
        grp = spbase[:, 5 * 2048:6 * 2048].reshape(C * 7, 128, WS)
        cat = np.concatenate([grp[:, :, :aw].reshape(C * 7, 128 * aw),
                              grp[:, :, aw:].reshape(C * 7,
                                                     128 * (WS - aw))],
                             axis=1)
        xspL_np = np.concatenate([
            cat.reshape(2, KH, 2048).transpose(1, 0, 2).reshape(KH, 4096),
            np.concatenate([np.ones(2048, np.float32),
                            np.zeros(2048, np.float32)])[None, :],
        ], axis=0).astype(E4M3)                              # [113, 4096]
        in_maps.append({"at": at_np, "xt": xt_np, "tt": tt_np,
                        "gt": gt_np, "bspec": bspec_np, "wsp": wsp_np,
                        "xsp": xsp_np, "xspL": xspL_np})

    nc = _build_main()
    kw = {}
    if bool(int(os.environ.get("KERNEL_TRACE", "0"))):
        d = os.environ.get("KERNEL_TRACE_DIR", "/tmp/ktrace") + "/main"
        os.makedirs(d, exist_ok=True)
        kw = dict(trace=True, tmpdir=d)
    res = run_bass_kernel_spmd(nc, in_maps, core_ids, **kw)
    global LAST_EXEC_NS
    LAST_EXEC_NS = res.exec_time_ns

    # ---- host: unshard + undo the x32 device scale; device columns are
    # completion-ordered [n, m2] ----
    out = np.empty((B, C, OUT_H, W), np.float32)
    for i in core_ids:
        o = (res.results[i]["out"].astype(np.float32)
             .reshape(128, NT, MT, WS, C).transpose(1, 4, 2, 0, 3)
             .reshape(B, C, OUT_H, WS))
        out[:, :, :, WS * i:WS * (i + 1)] = o * (1.0 / WSP_SCALE)
    return out


# revision 121
# speedup vs baseline: 1.0158x; 1.0016x over previous
"""Trainium2 Bass kernel for the FFTBlock problem (B=2, C=32, H=2688, W=128).

Math (reference):
  spatial  = relu(conv7x1_s7(x) + b_spatial)                        [B,C,384,W]
  spectral = irfft(relu(w_spectral @ rfft_concat(x) + b_spectral))  per (b,c,w)
  out = spatial + spectral

rfft/irfft along H are linear, so with F the real-ified rfft matrix and G the
irfft matrix (dead rows/cols dropped):
  spectral_col = G @ relu(A @ x_col + b),   A = w_spectral @ F  [384, 2688]

Device plan (W sharded 8 x 16 columns, one launch per core):
  GEMM1  conv[384, 1024] = A @ x_cols: 10 DoubleRow steps (K=256, both
         operands e4m3, x64 A scale) + one [64, 2, *] packed DR tail step.
  relu   (ACT/DVE, bias, n-split so GEMM2 starts per-half) -> f16 (x64;
         G absorbs the scale)
  GEMM2  spec[384, 1024] = (G*32/64) @ relu  (f16, 3 k-tiles); the psum is
         copied to f16 SBUF on the otherwise-idle ACT.
  spatial conv TRANSPOSED: stationary = x chunks [(c,t)=224+bias row,
         h'-block], moving = w_spatial (e4m3 x32, bias folded as an extra
         contraction row), DoubleRow -> psum [h', (b,w,co)] -- same layout as
         spec, so the add happens on-device and only ONE output is stored.
  out = relu(spatial psum) + spec(sbuf) in ONE DVE scalar_tensor_tensor per
         chunk, f16 x32-scaled (host divides by 32); three grouped stores.

Column order everywhere is (b, w, c) so the spatial conv's 32-channel output
blocks are contiguous in the spectral column space. All DRAM layouts are
pre-swizzled on host to partition-major so every DMA moves >=512B runs; the
load order is tuned so the PE never starves and the last input (xsp b1 m2=2)
has the shortest dependent chain.
"""

import os

import numpy as np
import ml_dtypes

import concourse.bacc as bacc
import concourse.mybir as mybir
import concourse.tile as tile
from concourse.bass_utils import run_bass_kernel_spmd
from concourse.alu_op_type import AluOpType

N_CORES = 8
B, C, H, W = 2, 32, 2688, 128
FREQ_IN = H // 2 + 1            # 1345
OUT_H = 384
FREQ_OUT = OUT_H // 2 + 1       # 193
MO = 2 * FREQ_OUT - 2           # 384 usable conv channels
WS = W // N_CORES               # 16 width columns per core
NCOL = B * WS * C               # 1024 spectral columns per core, (b, w, c)
NSP = B * OUT_H * WS            # 12288 spatial cols (b, h', w)
OLDK = H // 128                 # 21 k-tiles of 128
KD_DR = 10                      # full DoubleRow steps (old-k 0..2*KD_DR-1)
# the odd tail tile (old-k 20) runs as a [64, 2, *] DoubleRow step
MT = 3                          # 128-row m-tiles (G1 out / G2 out)
NT = 2                          # 512-col n halves; n == b
KH = 112                        # (c,t) DR half-pairs for spatial
KHB = KH + 1                    # +1 bias row

AT_SCALE = 64.0                 # fp8 range helper for A = w_spec @ F
WSP_SCALE = 32.0                # fp8 range helper for the tiny spatial weights

F32 = mybir.dt.float32
F16 = mybir.dt.float16
F8E4 = mybir.dt.float8e4
F8E3 = mybir.dt.float8e3
RELU = mybir.ActivationFunctionType.Relu
DR = mybir.MatmulPerfMode.DoubleRow
E4M3 = ml_dtypes.float8_e4m3
E3M4 = ml_dtypes.float8_e3m4

_cache = {}
LAST_EXEC_NS = None

# scheduling knobs (tuned against TimelineSim; see tuner.py)
CFG = {
    "warms": 0,           # p-state bridge dummies between G1 and G2
    "dve_relu01_first": False,  # relu(0,1) before relu(1,0) on DVE
    # mid-stream load order (xsp groups always follow); gt/wsp/tail ride
    # AFTER xt9 so GEMM1's last k-step lands ~0.8us earlier
    "load_order": "a0 x0 x1 x2 a3 x3 x4 bs x5 x6 a7 x7 x8 x9 tl gt ws",
    # w-split point of the last spatial group (piece A size in w's)
    "aw": 9,
}


def _dft_constants():
    """F [2688, 2688] (rfft, ortho, dead rows dropped) and G [384, 384]
    (irfft, ortho, dead cols dropped)."""
    if "F" in _cache:
        return _cache["F"], _cache["G"]
    Fc = np.fft.rfft(np.eye(H), axis=0, norm="ortho")       # [1345, 2688]
    F = np.concatenate([Fc.real, Fc.imag[1:FREQ_IN - 1]], axis=0)
    F = np.ascontiguousarray(F, dtype=np.float32)           # [2688, 2688]
    G_re = np.fft.irfft(np.eye(FREQ_OUT), n=OUT_H, axis=0, norm="ortho")
    G_im = np.fft.irfft(1j * np.eye(FREQ_OUT), n=OUT_H, axis=0, norm="ortho")
    G = np.concatenate([G_re, G_im[:, 1:FREQ_OUT - 1]], axis=1)
    G = np.ascontiguousarray(G, dtype=np.float32)           # [384, 384]
    _cache["F"] = F
    _cache["G"] = G
    return F, G


def _spec_keep_idx():
    keep_f = list(range(FREQ_IN)) + [FREQ_IN + k for k in range(1, FREQ_IN - 1)]
    keep_o = list(range(FREQ_OUT)) + [FREQ_OUT + k for k in range(1, FREQ_OUT - 1)]
    return np.array(keep_f), np.array(keep_o)


def _build_main():
    key = ("main", repr(sorted(CFG.items())))
    if key in _cache:
        return _cache[key]
    nc = bacc.Bacc("TRN2", target_bir_lowering=False, debug=False,
                   num_devices=N_CORES)
    at = nc.dram_tensor("at", [128, KD_DR * 2 * MO], F8E4,
                        kind="ExternalInput").ap()
    xt = nc.dram_tensor("xt", [128, KD_DR * 2 * NCOL], F8E4,
                        kind="ExternalInput").ap()
    # packed DR tail: att [64, 768] and xtt [64, 2048] merged in one tensor
    tt = nc.dram_tensor("tt", [64, 2 * MO + 2 * NCOL], F8E4,
                        kind="ExternalInput").ap()
    gt = nc.dram_tensor("gt", [128, MT * MO], F16, kind="ExternalInput").ap()
    bspec = nc.dram_tensor("bspec", [128, MT], F32, kind="ExternalInput").ap()
    wsp = nc.dram_tensor("wsp", [KHB, 2 * C], F8E4, kind="ExternalInput").ap()
    xsp = nc.dram_tensor("xsp", [KHB, 2 * NSP], F8E4,
                         kind="ExternalInput").ap()
    # last spatial group (b=1, m2=2) w-split into [12w | 4w] pieces so only
    # a small fadd trails the final input transfer
    xspL = nc.dram_tensor("xspL", [KHB, 2 * 2048], F8E4,
                          kind="ExternalInput").ap()
    out_d = nc.dram_tensor("out", [128, MT * NCOL], F16,
                           kind="ExternalOutput").ap()

    with tile.TileContext(nc) as tc:
        with tc.tile_pool(name="const", bufs=1) as cst, \
             tc.tile_pool(name="atp", bufs=1) as atp, \
             tc.tile_pool(name="xtp", bufs=1) as xtp, \
             tc.tile_pool(name="xspp", bufs=1) as xspp, \
             tc.tile_pool(name="relu", bufs=1) as rlp, \
             tc.tile_pool(name="outp", bufs=1) as outp, \
             tc.tile_pool(name="ps", bufs=1, space="PSUM") as psp, \
             tc.tile_pool(name="psw", bufs=1, space="PSUM") as psw:

            # ---------------- SBUF tiles ----------------
            AGRP = CFG.get("agrp", [(0, 3), (3, 7), (7, KD_DR)])
            at_g = {}
            for g0, g1 in AGRP:
                at_g[g0] = atp.tile([128, (g1 - g0) * 2 * MO], F8E4,
                                    tag=f"at{g0}", name=f"at{g0}")
            xt_t = [xtp.tile([128, 2 * NCOL], F8E4, tag=f"xt{s}",
                             name=f"xt{s}") for s in range(KD_DR)]
            tt_sb = xtp.tile([64, 2 * MO + 2 * NCOL], F8E4, tag="tt",
                             name="tt")
            gt_sb = cst.tile([128, MT * MO], F16, tag="gt", name="gt")
            bspec_sb = cst.tile([128, MT], F32, tag="bspec", name="bspec")
            wsp_sb = cst.tile([KHB, 2 * C], F8E4, tag="wsp", name="wsp")
            xsp_g = {(b, m2): xspp.tile([KHB, 2 * 2048], F8E4,
                                        tag=f"xsp{b}{m2}", name=f"xsp{b}{m2}")
                     for b in range(B) for m2 in range(MT)
                     if (b, m2) != (1, 2)}
            # split point: piece A = first AW w's, piece B = the rest
            AW = CFG.get("aw", 8)
            xspA = xspp.tile([KHB, 2 * 128 * AW], F8E4, tag="xspA",
                             name="xspA")
            xspB = xspp.tile([KHB, 2 * 128 * (WS - AW)], F8E4, tag="xspB",
                             name="xspB")
            # one tile per independently produced/consumed chunk -- the tile
            # framework tracks dependencies at TILE granularity, so shared
            # tiles would serialize unrelated producers/consumers
            relu_h = {(m, n): rlp.tile([128, 512], F16, tag=f"relu{m}{n}",
                                       name=f"relu{m}{n}")
                      for m in range(MT) for n in range(NT)}
            spec_sb = {(m2, n): rlp.tile([128, 512], F16, tag=f"sc{m2}{n}",
                                         name=f"sc{m2}{n}")
                       for m2 in range(MT) for n in range(NT)}
            # output SBUF grouped by store: A = all n0 chunks (ready ~same
            # time), B = (0,1)+(1,1), C = (2,1) alone -- three stores instead
            # of six halves the HWDGE generation serialization at the tail
            out_A = outp.tile([128, 1536], F16, tag="oA", name="oA")
            out_B = outp.tile([128, 1024], F16, tag="oB", name="oB")
            out_C = outp.tile([128, 512], F16, tag="oC", name="oC")

            def out_slice(m2, n):
                if n == 0:
                    return out_A[:, 512 * m2:512 * (m2 + 1)]
                if m2 == 2:
                    return out_C[:]
                return out_B[:, 512 * m2:512 * (m2 + 1)]

            def at_s(s, m):
                """DR stationary [*, 2, 128] for DR step s, m-tile m."""
                if s == KD_DR:
                    v = tt_sb[:, 0:2 * MO].rearrange("p (i m) -> p i m", i=2)
                else:
                    g0 = max(g for g, _ in AGRP if g <= s)
                    off = (s - g0) * 2 * MO
                    v = at_g[g0][:, off:off + 2 * MO].rearrange(
                        "p (i m) -> p i m", i=2)
                return v[:, :, 128 * m:128 * (m + 1)]

            def xt_s(s, n):
                if s == KD_DR:
                    v = tt_sb[:, 2 * MO:].rearrange("p (i n) -> p i n", i=2)
                else:
                    v = xt_t[s][:].rearrange("p (i n) -> p i n", i=2)
                return v[:, :, 512 * n:512 * (n + 1)]

            wsp_v = wsp_sb[:].rearrange("p (i m) -> p i m", i=2)
            gt_km = lambda k, m2: gt_sb[:, k * MO + 128 * m2:
                                        k * MO + 128 * (m2 + 1)]

            # ---------------- DMA emission (sync queue, in order) --------
            def load_at(g0):
                g1 = dict(AGRP)[g0]
                nc.sync.dma_start(at_g[g0][:],
                                  at[:, g0 * 2 * MO:g1 * 2 * MO])

            def load_xt(s):
                nc.sync.dma_start(xt_t[s][:],
                                  xt[:, s * 2 * NCOL:(s + 1) * 2 * NCOL])

            def load_tail():
                nc.sync.dma_start(tt_sb[:], tt[:])

            def load_xsp(b, m2):
                if (b, m2) == (1, 2):
                    srcL = xspL.rearrange("p (i n) -> p i n", i=2)
                    cut = 128 * AW
                    nc.sync.dma_start(
                        xspA[:].rearrange("p (i n) -> p i n", i=2),
                        srcL[:, :, 0:cut])
                    nc.sync.dma_start(
                        xspB[:].rearrange("p (i n) -> p i n", i=2),
                        srcL[:, :, cut:2048])
                    return
                src = xsp.rearrange("p (i n) -> p i n", i=2)[
                    :, :, b * (OUT_H * WS) + m2 * 2048:
                    b * (OUT_H * WS) + (m2 + 1) * 2048]
                dst = xsp_g[(b, m2)][:].rearrange("p (i n) -> p i n", i=2)
                nc.sync.dma_start(dst, src)

            # order tuned so the PE (starting at xt0+900ns sem prop) never
            # starves; small consts hide mid-stream behind big transfers; the
            # last input (xsp b1 m2=2) has the shortest dependent chain
            loaders = {
                # a0/a3/a7 historically named; they mean AGRP groups 0/1/2
                "a0": lambda: load_at(AGRP[0][0]),
                "a3": lambda: load_at(AGRP[1][0]),
                "a7": lambda: load_at(AGRP[2][0]) if len(AGRP) > 2 else None,
                "tl": load_tail,
                "gt": lambda: nc.sync.dma_start(gt_sb[:], gt[:]),
                "bs": lambda: nc.sync.dma_start(bspec_sb[:], bspec[:]),
                "ws": lambda: nc.sync.dma_start(wsp_sb[:], wsp[:]),
            }
            for tok in CFG["load_order"].split():
                if tok.startswith("x"):
                    load_xt(int(tok[1:]))
                else:
                    loaders[tok]()
            for b, m2 in CFG.get("xsp_order",
                                 [(0, 0), (0, 1), (0, 2),
                                  (1, 0), (1, 1), (1, 2)]):
                load_xsp(b, m2)

            # ---------------- compute ----------------
            ps1 = {(m, n): psp.tile([128, 512], F32, tag=f"g1m{m}n{n}",
                                    name=f"g1m{m}n{n}")
                   for m in range(MT) for n in range(NT)}

            def g1_step(s):
                # s == KD_DR is the [64, 2, *] packed tail step
                for n in range(NT):
                    for m in range(MT):
                        nc.tensor.matmul(ps1[(m, n)][:], at_s(s, m),
                                         xt_s(s, n), start=(s == 0),
                                         stop=(s == KD_DR), perf_mode=DR)

            sp_ps = {}

            def sp_chunk(b, m2):
                # transposed spatial conv: stationary = x slices, moving = w.
                # 16 tiny DR matmuls land [h'-block, (w,co)] directly in the
                # spectral output layout.
                j = b * MT + m2
                ps = psw.tile([128, 512], F32, tag=f"spp{j % 2}",
                              name=f"spp{j}")
                sp_ps[(b, m2)] = ps
                xv = xsp_g[(b, m2)][:].rearrange("p (i n) -> p i n", i=2)
                for w in range(WS):
                    nc.tensor.matmul(ps[:, 32 * w:32 * (w + 1)],
                                     xv[:, :, w::WS], wsp_v,
                                     start=True, stop=True, perf_mode=DR)

            def sp_piece(which):
                # (1,2) split: separate psum tiles so piece A's add never
                # waits piece B's (later) DMA
                if which == 0:
                    nw = AW
                    ps = psw.tile([128, 32 * nw], F32, tag="spp1",
                                  name="sppA")
                    xv = xspA[:].rearrange("p (i n) -> p i n", i=2)
                else:
                    nw = WS - AW
                    ps = psp.tile([128, 32 * nw], F32, tag="g1m0n0",
                                  name="sppB")
                    xv = xspB[:].rearrange("p (i n) -> p i n", i=2)
                sp_ps[("L", which)] = ps
                for w in range(nw):
                    nc.tensor.matmul(ps[:, 32 * w:32 * (w + 1)],
                                     xv[:, :, w::nw], wsp_v,
                                     start=True, stop=True, perf_mode=DR)

            # n-split relus let g2(n0) start as soon as the last k-step's
            # (m, n0) groups close; ACT/DVE checkerboard
            RELU_ACT = {(0, 0), (2, 0), (1, 1)}

            def relu_m(m, n):
                # relu1 scaled x64 (G absorbs /64); only ACT/DVE read PSUM
                if (m, n) in RELU_ACT:
                    nc.scalar.activation(relu_h[(m, n)][:], ps1[(m, n)][:],
                                         RELU, bias=bspec_sb[:, m:m + 1])
                else:
                    nc.vector.tensor_scalar(relu_h[(m, n)][:], ps1[(m, n)][:],
                                            bspec_sb[:, m:m + 1], 0.0,
                                            AluOpType.add, AluOpType.max)

            ps2 = {}

            def g2_n(n):
                for m2 in range(MT):
                    ps2[(m2, n)] = psp.tile([128, 512], F32,
                                            tag=f"g1m{m2}n{n}",
                                            name=f"g2m{m2}n{n}")
                if CFG.get("g2_m_major"):
                    # m-major: each m2 psum group closes ASAP, feeding the
                    # ACT spec-copy -> DVE fadd staircase earlier
                    for m2 in range(MT):
                        for k in range(MT):
                            nc.tensor.matmul(ps2[(m2, n)][:], gt_km(k, m2),
                                             relu_h[(k, n)][:],
                                             start=(k == 0),
                                             stop=(k == MT - 1))
                else:
                    for k in range(MT):
                        for m2 in range(MT):
                            nc.tensor.matmul(ps2[(m2, n)][:], gt_km(k, m2),
                                             relu_h[(k, n)][:],
                                             start=(k == 0),
                                             stop=(k == MT - 1))

            def copy_spec(m2, n):
                # spec psum -> f16 sbuf on the otherwise-idle ACT, well
                # before the spatial psum arrives -- keeps the tail chain to
                # a single DVE op per chunk
                nc.scalar.activation(spec_sb[(m2, n)][:], ps2[(m2, n)][:],
                                     mybir.ActivationFunctionType.Copy)

            def fadd(m2, n):
                # out = relu(spatial psum) + spec(sbuf) in ONE DVE op; both
                # branches are x32 scaled (the host divides the output by 32)
                nc.vector.scalar_tensor_tensor(
                    out_slice(m2, n), sp_ps[(n, m2)][:], 0.0,
                    spec_sb[(m2, n)][:], AluOpType.max, AluOpType.add)

            # out DRAM columns are completion-ordered: [n0m0 n0m1 n0m2
            # n1m0 n1m1 n1m2]; the host unshard accounts for this
            def store_A():
                nc.sync.dma_start(out_d[:, 0:1536], out_A[:])

            def store_B():
                nc.sync.dma_start(out_d[:, 1536:2560], out_B[:])

            def store_C():
                nc.sync.dma_start(out_d[:, 2560:3072], out_C[:])

            def pe_warm(i):
                # tiny dummy matmul bridging the relu-latency gap between
                # GEMM1's last k-step and GEMM2, so the PE p-state (and with
                # it GEMM2's 2.4GHz rate) survives the wait. Reads the LAST
                # xt tile so the scheduler cannot hoist it earlier.
                ps = psw.tile([1, 256], F32, tag="spp0", name=f"warm{i}")
                nc.tensor.matmul(ps[:], xt_t[KD_DR - 1][:, 0:1],
                                 xt_t[KD_DR - 1][:, 0:256],
                                 start=True, stop=True)

            # ---- PE order ----
            for s in range(KD_DR + 1):
                g1_step(s)
            relu_m(0, 0)
            if CFG["dve_relu01_first"]:
                relu_m(0, 1)
                relu_m(2, 0)
                relu_m(1, 0)
            else:
                relu_m(1, 0)
                relu_m(2, 0)
                relu_m(0, 1)
            relu_m(1, 1)
            relu_m(2, 1)
            for i in range(CFG["warms"]):
                pe_warm(i)
            g2_n(0)
            sp_chunk(0, 0)
            sp_chunk(0, 1)
            sp_chunk(0, 2)
            copy_spec(0, 0)
            copy_spec(1, 0)
            copy_spec(2, 0)
            g2_n(1)
            if CFG.get("sc21_first"):
                copy_spec(2, 1)
                copy_spec(0, 1)
                copy_spec(1, 1)
            else:
                copy_spec(0, 1)
                copy_spec(1, 1)
                copy_spec(2, 1)
            fadd(0, 0)
            fadd(1, 0)
            fadd(2, 0)
            store_A()
            sp_chunk(1, 0)
            fadd(0, 1)
            sp_chunk(1, 1)
            fadd(1, 1)
            store_B()
            acut = 32 * AW
            sp_piece(0)
            nc.vector.scalar_tensor_tensor(
                out_C[:, 0:acut], sp_ps[("L", 0)][:], 0.0,
                spec_sb[(2, 1)][:, 0:acut], AluOpType.max, AluOpType.add)
            sp_piece(1)
            nc.vector.scalar_tensor_tensor(
                out_C[:, acut:512], sp_ps[("L", 1)][:], 0.0,
                spec_sb[(2, 1)][:, acut:512], AluOpType.max, AluOpType.add)
            store_C()

    nc.compile()
    _cache["main"] = nc
    return nc


def _host_prep(x, w_spatial, b_spatial, w_spectral, b_spectral):
    """Shared (weight) swizzles."""
    F, G = _dft_constants()
    keep_f, keep_o = _spec_keep_idx()

    A = w_spectral[keep_o][:, keep_f] @ F                    # [384, 2688]
    arrA = np.ascontiguousarray((A * AT_SCALE).T)            # [2688, 384]
    hcut = KD_DR * 256
    at_np = np.ascontiguousarray(
        arrA[:hcut].reshape(KD_DR * 2, 128, MO).transpose(1, 0, 2)
        .reshape(128, KD_DR * 2 * MO)).astype(E4M3)
    att_np = (arrA[hcut:].reshape(2, 64, MO).transpose(1, 0, 2)
              .reshape(64, 2 * MO)).astype(E4M3)
    # gt absorbs both the relu1 x64 scale and the x32 output scale (the
    # device emits 32*(spatial+spectral); the host divides by 32)
    gt_np = np.ascontiguousarray(
        (G.T * (WSP_SCALE / AT_SCALE)).reshape(MT, 128, MO).transpose(1, 0, 2)
        .reshape(128, MT * MO)).astype(np.float16)
    bspec_np = np.ascontiguousarray(
        (b_spectral[keep_o] * AT_SCALE).reshape(MT, 128).T).astype(np.float32)
    wbase = (w_spatial[:, :, :, 0].transpose(1, 2, 0).reshape(C * 7, C)
             * WSP_SCALE)
    wsp_np = np.concatenate([
        wbase.reshape(2, KH, C).transpose(1, 0, 2).reshape(KH, 2 * C),
        np.concatenate([b_spatial * WSP_SCALE, np.zeros(C)])[None, :],
    ], axis=0).astype(E4M3)                                  # [113, 64]
    return at_np, att_np, gt_np, bspec_np, wsp_np


def kernel(x, w_spatial, b_spatial, w_spectral, b_spectral):
    x = np.ascontiguousarray(x, dtype=np.float32)
    w_spatial = np.asarray(w_spatial, dtype=np.float32)
    b_spatial = np.asarray(b_spatial, dtype=np.float32)
    w_spectral = np.asarray(w_spectral, dtype=np.float32)
    b_spectral = np.asarray(b_spectral, dtype=np.float32)

    at_np, att_np, gt_np, bspec_np, wsp_np = _host_prep(
        x, w_spatial, b_spatial, w_spectral, b_spectral)
    core_ids = list(range(N_CORES))
    hcut = KD_DR * 256

    in_maps = []
    for i in core_ids:
        xs = x[:, :, :, WS * i:WS * (i + 1)]                 # [B, C, H, WS]
        arr = xs.transpose(2, 0, 3, 1).reshape(H, NCOL)      # [H, (b,w,c)]
        xt_np = np.ascontiguousarray(
            arr[:hcut].reshape(KD_DR * 2, 128, NCOL).transpose(1, 0, 2)
            .reshape(128, KD_DR * 2 * NCOL)).astype(E4M3)
        xtt_np = (arr[hcut:].reshape(2, 64, NCOL).transpose(1, 0, 2)
                  .reshape(64, 2 * NCOL)).astype(E4M3)
        tt_np = np.ascontiguousarray(
            np.concatenate([att_np, xtt_np], axis=1))        # [64, 2816]
        spbase = (xs.reshape(B, C, OUT_H, 7, WS).transpose(1, 3, 0, 2, 4)
                  .reshape(C * 7, NSP))                      # [(c,t),(b,h',w)]
        xsp_np = np.concatenate([
            spbase.reshape(2, KH, NSP).transpose(1, 0, 2).reshape(KH, 2 * NSP),
            np.concatenate([np.ones(NSP, np.float32),
                            np.zeros(NSP, np.float32)])[None, :],
        ], axis=0).astype(E4M3)                              # [113, 2*NSP]
        # last group (b=1, m2=2) w-split [AW | WS-AW], cols (w-block, h', w)
        aw = CFG.get("aw", 8)
        grp = spbase[:, 5 * 2048:6 * 2048].reshape(C * 7, 128, WS)
        cat = np.concatenate([grp[:, :, :aw].reshape(C * 7, 128 * aw),
                              grp[:, :, aw:].reshape(C * 7,
                                                     128 * (WS - aw))],
                             axis=1)
        xspL_np = np.concatenate([
            cat.reshape(2, KH, 2048).transpose(1, 0, 2).reshape(KH, 4096),
            np.concatenate([np.ones(2048, np.float32),
                            np.zeros(2048, np.float32)])[None, :],
        ], axis=0).astype(E4M3)                              # [113, 4096]
        in_maps.append({"at": at_np, "xt": xt_np, "tt": tt_np,
                        "gt": gt_np, "bspec": bspec_np, "wsp": wsp_np,
                        "xsp": xsp_np, "xspL": xspL_np})

    nc = _build_main()
    kw = {}
    if bool(int(os.environ.get("KERNEL_TRACE", "0"))):
        d = os.environ.get("KERNEL_TRACE_DIR", "/tmp/ktrace") + "/main"
        os.makedirs(d, exist_ok=True)
        kw = dict(trace=True, tmpdir=d)
    res = run_bass_kernel_spmd(nc, in_maps, core_ids, **kw)
    global LAST_EXEC_NS
    LAST_EXEC_NS = res.exec_time_ns

    # ---- host: unshard + undo the x32 device scale; device columns are
    # completion-ordered [n, m2] ----
    out = np.empty((B, C, OUT_H, W), np.float32)
    for i in core_ids:
        o = (res.results[i]["out"].astype(np.float32)
             .reshape(128, NT, MT, WS, C).transpose(1, 4, 2, 0, 3)
             .reshape(B, C, OUT_H, WS))
        out[:, :, :, WS * i:WS * (i + 1)] = o * (1.0 / WSP_SCALE)
    return out
